# revision 1
# baseline (speedup 1.0000x reference)
"""BigBird sparse attention kernel for 8 Trainium2 NeuronCores.

Sharding: token-parallel. B=2 batches x 4 chunks of 1024 local tokens each
-> 8 cores. Each core receives a transposed x-slice [D=1024, 1282] whose
columns are [g0, g1, 10 window blocks of 128 tokens] (blocks 8j-1 .. 8j+8,
zero-padded outside [0, 32)). The core computes:
  - q/k projections in transposed layout [f, tok] (fp32r matmuls)
  - v projection in [tok, f] layout
  - 3-block sliding-window attention with scores kept transposed [kt, q]
    (exp'd probabilities feed P.V matmuls directly, denominator rides as a
    ones column in the V stationary)
  - attention of local tokens to the 2 global tokens (separate softmax)
  - flash-style partial stats (sum-exp, weighted V) of the 2 global query
    tokens against the core's local keys -> combined on host
  - output projection + bias for its 1024 local tokens
Host assembles the 8 slices, and computes the 2 global output rows per
batch exactly in numpy from the shipped partials.
"""

import numpy as np
import ml_dtypes

import concourse.bass as bass
import concourse.mybir as mybir
import concourse.tile as tile
from concourse import bacc
from concourse.bass_utils import run_bass_kernel_spmd

F32 = mybir.dt.float32
F32R = mybir.dt.float32r
BF16 = mybir.dt.bfloat16
AF = mybir.ActivationFunctionType
BF = ml_dtypes.bfloat16

D_MODEL = 1024
H = 16
DK = 64
BS = 128
B = 2
T = 4098
NB = 32            # global 128-blocks of local tokens
NW = 10            # window blocks per core (8 local + 2 halo)
TOKS = 2 + NW * BS # x-slice columns
LQ0 = 2 + BS       # first local-q column
SCALE = 1.0 / np.sqrt(DK)

# token chunks for the projection moving dim (all >=256 for fp32r speed)
CHUNKS = [(0, 512), (512, 512), (1024, 258)]


def C(t):
    return 2 + BS * t


# P.V accumulation schedule per psum bank: (t, qstart, nblocks, start, stop)
# bank 0 covers q window-positions 1..4, bank 1 covers 5..8.
PV_SCHED = [
    [(2, 1, 3, True, False), (3, 2, 2, False, False), (3, 4, 1, False, False),
     (0, 1, 1, False, False), (1, 1, 2, False, False), (4, 3, 2, False, False),
     (5, 4, 1, False, True)],
    [(6, 5, 3, True, False), (7, 6, 2, False, False), (7, 8, 1, False, False),
     (4, 5, 1, False, False), (5, 5, 2, False, False), (8, 7, 2, False, False),
     (9, 8, 1, False, True)],
]


def ptcol(t, qpos):
    # column of (window-block t, q window-position qpos) in the pt tensor
    return 384 * t + 128 * (qpos - (t - 1))


def build_kernel(nc):
    xt = nc.dram_tensor("xt", [D_MODEL, TOKS], F32, kind="ExternalInput").ap()
    wq = nc.dram_tensor("wq", [8, 8, 128, 128], F32, kind="ExternalInput").ap()
    wk = nc.dram_tensor("wk", [8, 8, 128, 128], F32, kind="ExternalInput").ap()
    wv = nc.dram_tensor("wv", [2, 8, 128, 512], F32, kind="ExternalInput").ap()
    wo = nc.dram_tensor("wo", [8, 8, 128, 128], BF16, kind="ExternalInput").ap()
    bo = nc.dram_tensor("bo", [D_MODEL], F32, kind="ExternalInput").ap()
    maskl = nc.dram_tensor("maskl", [128, 1], BF16, kind="ExternalInput").ap()
    maskr = nc.dram_tensor("maskr", [128, 1], BF16, kind="ExternalInput").ap()
    outt = nc.dram_tensor("outt", [D_MODEL, 1024], F32, kind="ExternalOutput").ap()
    gstats = nc.dram_tensor("gstats", [65, 32], F32, kind="ExternalOutput").ap()
    import os as _os
    dbg = None
    if _os.environ.get("BB_DEBUG"):
        dbg = nc.dram_tensor("dbg_at", [D_MODEL, 1024], BF16,
                             kind="ExternalOutput").ap()

    with tile.TileContext(nc) as tc:
        with (
            tc.tile_pool(name="pc", bufs=1) as pc,
            tc.tile_pool(name="px", bufs=1) as px,
            tc.tile_pool(name="pqk", bufs=1) as pqk,
            tc.tile_pool(name="pv", bufs=1) as pvp,
            tc.tile_pool(name="pwv", bufs=1) as pwv,
            tc.tile_pool(name="pw", bufs=6) as pw,
            tc.tile_pool(name="pat", bufs=1) as pat,
            tc.tile_pool(name="ppt", bufs=2) as ppt,
            tc.tile_pool(name="psm", bufs=2) as psm,
            tc.tile_pool(name="pout", bufs=2) as pout,
            tc.tile_pool(name="pps", bufs=8, space="PSUM") as pps,
        ):
            # ---- constants ----
            bo_sb = pc.tile([128, 8], F32, tag="bo")
            nc.sync.dma_start(bo_sb[:], bo.rearrange("(t p) -> p t", p=128))
            ml_sb = pc.tile([128, 1], BF16, tag="ml")
            mr_sb = pc.tile([128, 1], BF16, tag="mr")
            nc.sync.dma_start(ml_sb[:], maskl)
            nc.sync.dma_start(mr_sb[:], maskr)
            ones32 = pc.tile([2, 32], BF16, tag="ones32")
            nc.vector.memset(ones32[:], 0.0)
            nc.vector.memset(ones32[:, 0:1], 1.0)
            gst = pc.tile([65, 32], F32, tag="gst")

            # ---- x slice, transposed, resident ----
            xts = []
            for d in range(8):
                xd = px.tile([128, TOKS], F32R, tag=f"xt{d}")
                rows = xt[128 * d:128 * (d + 1), :].bitcast(F32R)
                nc.sync.dma_start(xd[:, 0:512], rows[:, 0:512])
                nc.sync.dma_start(xd[:, 512:TOKS], rows[:, 512:TOKS])
                xts.append(xd)

            at_sb = [pat.tile([128, 1024], BF16, tag=f"at{f}", name=f"at{f}")
                     for f in range(8)]

            def emit_qk_proj(pss, half, qk_tiles):
                for pname, wdram in (("q", wq), ("k", wk)):
                    osbs, psjs = [], []
                    for i2 in range(2):
                        i = 2 * half + i2
                        osb = pqk.tile([128, TOKS], F32R, tag=f"qk{pname}{i}",
                                       name=f"qk{pname}{i}")
                        qk_tiles[(pname, i)] = osb
                        osbs.append(osb)
                        psjs.append([pps.tile([128, cn], F32, tag="ps1",
                                              name=f"pj{i2}_{c}")
                                     for c, (c0, cn) in enumerate(CHUNKS)])
                    for d in range(8):
                        wt = pw.tile([128, 256], F32R, tag="w")
                        ft0 = 4 * pss + 2 * half
                        src = bass.AP(wdram.tensor,
                                      wdram[ft0, d].offset,
                                      [[128, 128], [8 * 128 * 128, 2], [1, 128]])
                        nc.sync.dma_start(wt[:], src.bitcast(F32R))
                        for i2 in range(2):
                            for c, (c0, cn) in enumerate(CHUNKS):
                                nc.tensor.matmul(
                                    psjs[i2][c][:, :cn],
                                    wt[:, 128 * i2:128 * i2 + 128],
                                    xts[d][:, c0:c0 + cn],
                                    start=(d == 0), stop=(d == 7))
                    for i2 in range(2):
                        for c, (c0, cn) in enumerate(CHUNKS):
                            if pname == "q":
                                nc.scalar.mul(osbs[i2][:, c0:c0 + cn],
                                              psjs[i2][c][:, :cn], SCALE)
                            else:
                                nc.scalar.copy(osbs[i2][:, c0:c0 + cn],
                                               psjs[i2][c][:, :cn])

            def emit_v_proj(pss):
                wv_sb = []
                for d in range(8):
                    wvd = pwv.tile([128, 512], F32R, tag=f"wv{d}", name=f"wv{d}")
                    nc.sync.dma_start(wvd[:], wv[pss, d].bitcast(F32R))
                    wv_sb.append(wvd)
                v96 = []
                for tb in range(NW):
                    pv_ps = pps.tile([128, 512], F32, tag="ps1", name="pv_ps")
                    for d in range(8):
                        nc.tensor.matmul(pv_ps[:], xts[d][:, C(tb):C(tb) + 128],
                                         wv_sb[d][:], start=(d == 0), stop=(d == 7))
                    vt = pvp.tile([128, 8 * 96], BF16, tag=f"v96_{tb}",
                                  name=f"v96_{tb}")
                    pstep = vt.ap[0][0]
                    dst = bass.AP(vt.tensor, vt[:].offset,
                                  [[pstep, 128], [96, 8], [1, 64]])
                    src = bass.AP(pv_ps.tensor, pv_ps[:].offset,
                                  [[pv_ps.ap[0][0], 128], [64, 8], [1, 64]])
                    nc.vector.tensor_copy(dst, src)
                    onesap = bass.AP(vt.tensor, vt[:].offset + 64,
                                     [[pstep, 128], [96, 8], [1, 1]])
                    nc.vector.memset(onesap, 1.0)
                    zap = bass.AP(vt.tensor, vt[:].offset + 65,
                                  [[pstep, 128], [96, 8], [1, 31]])
                    nc.vector.memset(zap, 0.0)
                    v96.append(vt)
                pvg = pps.tile([2, 512], F32, tag="ps1", name="pvg")
                for d in range(8):
                    nc.tensor.matmul(pvg[:], xts[d][:, 0:2], wv_sb[d][:],
                                     start=(d == 0), stop=(d == 7))
                vg_sb = pvp.tile([2, 8 * 96], BF16, tag="vg", name="vg")
                gstep = vg_sb.ap[0][0]
                gdst = bass.AP(vg_sb.tensor, vg_sb[:].offset,
                               [[gstep, 2], [96, 8], [1, 64]])
                gsrc = bass.AP(pvg.tensor, pvg[:].offset,
                               [[pvg.ap[0][0], 2], [64, 8], [1, 64]])
                nc.scalar.copy(gdst, gsrc)
                g1 = bass.AP(vg_sb.tensor, vg_sb[:].offset + 64,
                             [[gstep, 2], [96, 8], [1, 1]])
                nc.vector.memset(g1, 1.0)
                g0 = bass.AP(vg_sb.tensor, vg_sb[:].offset + 65,
                             [[gstep, 2], [96, 8], [1, 31]])
                nc.vector.memset(g0, 0.0)
                return v96, vg_sb

            def emit_head(h, qk_tiles, v96, vg_sb):
                hl = h % 8
                r0 = 64 * (hl % 2)
                qh = qk_tiles[("q", hl // 2)][r0:r0 + 64, :]
                kh = qk_tiles[("k", hl // 2)][r0:r0 + 64, :]

                # xg scores + exp first so ACT serves them before the
                # score exps (oxg/wv matmuls then never wait on ACT backlog)
                pxg = psm.tile([2, 1024], BF16, tag="pxg", name="pxg", bufs=3)
                for c in range(2):
                    ps_xg = pps.tile([2, 512], F32, tag="ps1", name="ps_xg")
                    nc.tensor.matmul(ps_xg[:], kh[:, 0:2],
                                     qh[:, LQ0 + 512 * c:LQ0 + 512 * c + 512],
                                     start=True, stop=True)
                    nc.scalar.activation(pxg[:, 512 * c:512 * c + 512], ps_xg[:],
                                         AF.Exp)
                pt = ppt.tile([128, 3840], BF16, tag="pt", name="pt")
                psg = pps.tile([128, 16], F32, tag="ps1", name="psg")
                for t in range(1, 9):
                    nc.tensor.matmul(psg[:, 2 * (t - 1):2 * t],
                                     kh[:, C(t):C(t) + 128], qh[:, 0:2],
                                     start=(t == 1), stop=(t == 8))
                pg = psm.tile([128, 16], BF16, tag="pgsb", name="pg", bufs=3)
                nc.scalar.activation(pg[:], psg[:], AF.Exp)
                for t in range(NW):
                    qlo, qhi = max(t - 1, 1), min(t + 1, 8)
                    n = (qhi - qlo + 1) * 128
                    ps_s = pps.tile([128, 384], F32, tag="ps1", name="ps_s")
                    nc.tensor.matmul(ps_s[:, :n], kh[:, C(t):C(t) + 128],
                                     qh[:, C(qlo):C(qlo) + n],
                                     start=True, stop=True)
                    col = ptcol(t, qlo)
                    nc.scalar.activation(pt[:, col:col + n], ps_s[:, :n], AF.Exp)
                    if t == 0:
                        nc.gpsimd.tensor_mul(pt[:, col:col + n], pt[:, col:col + n],
                                             ml_sb[:].to_broadcast((128, n)))
                    if t == NW - 1:
                        nc.gpsimd.tensor_mul(pt[:, col:col + n], pt[:, col:col + n],
                                             mr_sb[:].to_broadcast((128, n)))
                ps_ob = [pps.tile([96, 512], F32, tag="ps1", name=f"po{bank}")
                         for bank in range(2)]
                for bank in range(2):
                    for (t, qs, nb, st, sp) in PV_SCHED[bank]:
                        c0 = 128 * (qs - 1) - 512 * bank
                        nc.tensor.matmul(
                            ps_ob[bank][:, c0:c0 + 128 * nb],
                            v96[t][:, 96 * hl:96 * hl + 96],
                            pt[:, ptcol(t, qs):ptcol(t, qs) + 128 * nb],
                            start=st, stop=sp)
                ps_wv = pps.tile([96, 2], F32, tag="ps1", name="ps_wv")
                for t in range(1, 9):
                    nc.tensor.matmul(ps_wv[:], v96[t][:, 96 * hl:96 * hl + 96],
                                     pg[:, 2 * (t - 1):2 * t],
                                     start=(t == 1), stop=(t == 8))
                ps_oxb = [pps.tile([96, 512], F32, tag="ps1", name=f"pox{c}")
                          for c in range(2)]
                for c in range(2):
                    nc.tensor.matmul(ps_oxb[c][:],
                                     vg_sb[:, 96 * hl:96 * hl + 96],
                                     pxg[:, 512 * c:512 * c + 512],
                                     start=True, stop=True)

                bl = psm.tile([64, 1024], F32, tag="bl", name="bl")
                bxg = psm.tile([64, 1024], F32, tag="bxg", name="bxg")
                for bank in range(2):
                    sl = slice(512 * bank, 512 * bank + 512)
                    nc.vector.stream_shuffle(bl[0:32, sl], ps_ob[bank][64:96, :],
                                             [0] * 32)
                    nc.vector.stream_shuffle(bl[32:64, sl], ps_ob[bank][64:96, :],
                                             [0] * 32)
                    nc.vector.stream_shuffle(bxg[0:32, sl], ps_oxb[bank][64:96, :],
                                             [0] * 32)
                    nc.vector.stream_shuffle(bxg[32:64, sl], ps_oxb[bank][64:96, :],
                                             [0] * 32)
                cp_o = psm.tile([64, 1024], BF16, tag="cpo", name="cp_o")
                cp_ox = psm.tile([64, 1024], BF16, tag="cpox", name="cp_ox")
                for bank in range(2):
                    sl = slice(512 * bank, 512 * bank + 512)
                    nc.vector.tensor_copy(cp_o[:, sl], ps_ob[bank][0:64, :])
                    nc.scalar.copy(cp_ox[:, sl], ps_oxb[bank][0:64, :])
                nc.vector.reciprocal(bl[:], bl[:])
                nc.vector.reciprocal(bxg[:], bxg[:])
                tmp = psm.tile([64, 1024], F32, tag="tmp", name="tmp")
                tmp2 = psm.tile([64, 1024], F32, tag="tmp2", name="tmp2")
                nc.gpsimd.tensor_mul(tmp[:], cp_o[:], bl[:])
                nc.gpsimd.tensor_mul(tmp2[:], cp_ox[:], bxg[:])
                nc.gpsimd.tensor_add(at_sb[h // 2][r0:r0 + 64, :], tmp[:], tmp2[:])
                nc.scalar.copy(gst[:, 2 * h:2 * h + 2], ps_wv[0:65, :])

            # software-pipelined emission: pass-B q/k projections interleave
            # with pass-A attention head groups (PE executes in program order)
            qk0, qk1 = {}, {}
            emit_qk_proj(0, 0, qk0)
            emit_qk_proj(0, 1, qk0)
            v96_0, vg0 = emit_v_proj(0)
            for h in range(0, 4):
                emit_head(h, qk0, v96_0, vg0)
            emit_qk_proj(1, 0, qk1)
            for h in range(4, 8):
                emit_head(h, qk0, v96_0, vg0)
            emit_qk_proj(1, 1, qk1)
            v96_1, vg1 = emit_v_proj(1)
            for h in range(8, 16):
                emit_head(h, qk1, v96_1, vg1)

            # ================= output projection =================
            # prefetch the first weight tiles before the barrier so their DMAs
            # land during the attention tail
            wot_pre = []
            for m in range(2):
                wotp = pw.tile([128, 1024], BF16, tag="wo", bufs=3,
                               name=f"wot{m}")
                wsrc = bass.AP(wo.tensor, wo[m, 0].offset,
                               [[128, 128], [128 * 128, 8], [1, 128]])
                nc.sync.dma_start(wotp[:], wsrc)
                wot_pre.append(wotp)
            tc.no_sync_barrier()
            for m in range(8):
                ps_op = [pps.tile([128, 512], F32, tag="ps1", name=f"pop{c}")
                         for c in range(2)]
                if m < 2:
                    wot = wot_pre[m]
                else:
                    wot = pw.tile([128, 1024], BF16, tag="wo", bufs=3)
                    wsrc = bass.AP(wo.tensor, wo[m, 0].offset,
                                   [[128, 128], [128 * 128, 8], [1, 128]])
                    nc.sync.dma_start(wot[:], wsrc)
                for f in range(8):
                    for c in range(2):
                        nc.tensor.matmul(ps_op[c][:], wot[:, 128 * f:128 * f + 128],
                                         at_sb[f][:, 512 * c:512 * c + 512],
                                         start=(f == 0), stop=(f == 7))
                for c in range(2):
                    ot = pout.tile([128, 512], F32, tag="ot")
                    nc.scalar.activation(ot[:], ps_op[c][:], AF.Identity,
                                         bias=bo_sb[:, m:m + 1])
                    nc.sync.dma_start(outt[128 * m:128 * (m + 1),
                                           512 * c:512 * c + 512], ot[:])
            nc.sync.dma_start(gstats, gst[:])
            if dbg is not None:
                for f in range(8):
                    nc.sync.dma_start(dbg[128 * f:128 * (f + 1), :], at_sb[f][:])
    return nc


_NC_CACHE = {}
LAST = {}


def get_nc():
    if "nc" not in _NC_CACHE:
        nc = bacc.Bacc("TRN2", target_bir_lowering=False, debug=False, num_devices=8)
        build_kernel(nc)
        nc.compile()
        _NC_CACHE["nc"] = nc
    return _NC_CACHE["nc"]


def make_inputs(x, Wq, Wk, Wv, Wo, bo):
    """Build the 8 per-core input maps (all host-side numpy)."""
    x = np.asarray(x, np.float32)
    Wq = np.asarray(Wq, np.float32)
    Wk = np.asarray(Wk, np.float32)
    Wv = np.asarray(Wv, np.float32)
    Wo = np.asarray(Wo, np.float32)
    bo = np.asarray(bo, np.float32)

    wq_r = np.ascontiguousarray(
        Wq.T.reshape(8, 128, 8, 128).transpose(2, 0, 1, 3))  # [ft, d, 128d, 128f]
    wk_r = np.ascontiguousarray(Wk.T.reshape(8, 128, 8, 128).transpose(2, 0, 1, 3))
    wv_r = np.ascontiguousarray(
        Wv.T.reshape(8, 128, 2, 512).transpose(2, 0, 1, 3))  # [fh, d, 128d, 512f]
    wo_r = np.ascontiguousarray(
        Wo.T.reshape(8, 128, 8, 128).transpose(2, 0, 1, 3)).astype(BF)
    # wo_r[m, f, i, j] must be Wo[128m+j, 128f+i] = Wo.T[128f+i, 128m+j]

    ones = np.ones((128, 1), BF)
    zeros = np.zeros((128, 1), BF)
    in_maps = []
    for core in range(8):
        b, j = divmod(core, 4)
        xs = np.zeros((TOKS, D_MODEL), np.float32)
        xs[0] = x[b, 0]
        xs[1] = x[b, T - 1]
        for w in range(NW):
            gb = 8 * j - 1 + w
            if 0 <= gb < NB:
                xs[2 + 128 * w:2 + 128 * (w + 1)] = x[b, 1 + 128 * gb:1 + 128 * (gb + 1)]
        in_maps.append({
            "xt": np.ascontiguousarray(xs.T),
            "wq": wq_r, "wk": wk_r, "wv": wv_r, "wo": wo_r, "bo": bo,
            "maskl": zeros if j == 0 else ones,
            "maskr": zeros if j == 3 else ones,
        })
    return in_maps


def assemble_output(results, x, Wq, Wk, Wv, Wo, bo):
    x = np.asarray(x, np.float32)
    out = np.empty((B, T, D_MODEL), np.float32)
    for core in range(8):
        b, j = divmod(core, 4)
        out[b, 1 + 1024 * j:1 + 1024 * (j + 1), :] = results[core]["outt"].T

    # global token rows, exact on host
    xg = x[:, [0, T - 1], :]                      # [B, 2, D]
    qg = (xg @ Wq.T).reshape(B, 2, H, DK) * SCALE  # [B, 2, H, DK]
    kg = (xg @ Wk.T).reshape(B, 2, H, DK)
    vg = (xg @ Wv.T).reshape(B, 2, H, DK)
    for b in range(B):
        se = np.zeros((H, 2))
        wvs = np.zeros((H, 2, DK))
        for j in range(4):
            g = results[4 * b + j]["gstats"]  # [65, 32]
            for h in range(H):
                for gi in range(2):
                    se[h, gi] += g[64, 2 * h + gi]
                    wvs[h, gi] += g[0:64, 2 * h + gi]
        # add the global-key terms: scores qg . kg
        sgg = np.einsum("ghd,fhd->hgf", qg[b], kg[b])  # [H, 2g(query), 2f(key)]
        egg = np.exp(sgg)
        num = wvs + np.einsum("hgf,fhd->hgd", egg, vg[b])
        den = se + egg.sum(-1)
        og = num / den[..., None]                  # [H, 2, DK]
        for gi, trow in ((0, 0), (1, T - 1)):
            row = og[:, gi, :].reshape(H * DK)
            out[b, trow] = row @ Wo.T + bo
    return out


def kernel(x, Wq, Wk, Wv, Wo, bo):
    nc = get_nc()
    in_maps = make_inputs(x, Wq, Wk, Wv, Wo, bo)
    res = run_bass_kernel_spmd(nc, in_maps, core_ids=list(range(8)))
    LAST["res"] = res
    results = [{k: np.asarray(v) for k, v in r.items()} for r in res.results]
    return assemble_output(results, x, Wq, Wk, Wv, Wo, bo)



# revision 3
# speedup vs baseline: 1.7345x; 1.7345x over previous
"""BigBird sparse attention kernel for 8 Trainium2 NeuronCores.

Sharding: token-parallel. B=2 batches x 4 chunks of 1024 local tokens each
-> 8 cores. Each core receives a transposed x-slice [D=1024, 1282] whose
columns are [g0, g1, 10 window blocks of 128 tokens] (blocks 8j-1 .. 8j+8,
zero-padded outside [0, 32)). The core computes:
  - q/k projections in transposed layout [f, tok] (fp32r matmuls, bf16 out)
  - v projection in [tok, f] layout with a ones-column per head
  - 3-block sliding-window attention: scores kept transposed [kt, q], exp'd
    to bf16 probabilities; the P.V matmul is FLIPPED (stationary = P block,
    moving = V||ones) so each 128-token q block lands in PSUM as
    [q, 64 v-cols + denominator] with the softmax denominator per-partition
  - attention of local tokens to the 2 global tokens (separate softmax,
    same flipped layout) -> normalize both with per-partition reciprocals,
    combine, transpose back to [feat, tok] on the PE array
  - flash-style partial stats (sum-exp, weighted V) of the 2 global query
    tokens against the core's local keys -> combined on host
  - output projection + bias for its 1024 local tokens
Host assembles the 8 slices, and computes the 2 global output rows per
batch exactly in numpy from the shipped partials.
"""

import numpy as np
import ml_dtypes

import concourse.bass as bass
import concourse.mybir as mybir
import concourse.tile as tile
from concourse import bacc
from concourse.bass_utils import run_bass_kernel_spmd

F32 = mybir.dt.float32
F32R = mybir.dt.float32r
BF16 = mybir.dt.bfloat16
AF = mybir.ActivationFunctionType
BF = ml_dtypes.bfloat16

D_MODEL = 1024
H = 16
DK = 64
BS = 128
B = 2
T = 4098
NB = 32            # global 128-blocks of local tokens
NW = 10            # window blocks per core (8 local + 2 halo)
TOKS = 2 + NW * BS # x-slice columns
LQ0 = 2 + BS       # first local-q column
SCALE = 1.0 / np.sqrt(DK)

# token chunks for the projection moving dim (all >=256 for fp32r speed)
CHUNKS = [(0, 512), (512, 512), (1024, 258)]

# pt region start per window block t (regions sized by the q-window width)
PT_START = [0, 128, 384, 768, 1152, 1536, 1920, 2304, 2688, 2944]
PT_COLS = 3072


def C(t):
    return 2 + BS * t


def qlo(t):
    return max(t - 1, 1)


def ptcol(t, qs):
    # column of (window-block t, q window-position qs) in the pt tensor
    return PT_START[t] + 128 * (qs - qlo(t))


def build_kernel(nc):
    xt = nc.dram_tensor("xt", [D_MODEL, TOKS], F32, kind="ExternalInput").ap()
    wq = nc.dram_tensor("wq", [8, 8, 128, 128], F32, kind="ExternalInput").ap()
    wk = nc.dram_tensor("wk", [8, 8, 128, 128], F32, kind="ExternalInput").ap()
    wv = nc.dram_tensor("wv", [2, 8, 128, 512], F32, kind="ExternalInput").ap()
    wo = nc.dram_tensor("wo", [8, 8, 128, 128], BF16, kind="ExternalInput").ap()
    bo = nc.dram_tensor("bo", [D_MODEL], F32, kind="ExternalInput").ap()
    maskl = nc.dram_tensor("maskl", [128, 1], BF16, kind="ExternalInput").ap()
    maskr = nc.dram_tensor("maskr", [128, 1], BF16, kind="ExternalInput").ap()
    ident = nc.dram_tensor("ident", [128, 128], BF16, kind="ExternalInput").ap()
    outt = nc.dram_tensor("outt", [D_MODEL, 1024], F32, kind="ExternalOutput").ap()
    gstats = nc.dram_tensor("gstats", [65, 32], F32, kind="ExternalOutput").ap()
    import os as _os
    dbg = None
    if _os.environ.get("BB_DEBUG"):
        dbg = nc.dram_tensor("dbg_at", [D_MODEL, 1024], BF16,
                             kind="ExternalOutput").ap()

    with tile.TileContext(nc) as tc:
        with (
            tc.tile_pool(name="pc", bufs=1) as pc,
            tc.tile_pool(name="px", bufs=1) as px,
            tc.tile_pool(name="pqk", bufs=1) as pqk,
            tc.tile_pool(name="pv", bufs=1) as pvp,
            tc.tile_pool(name="pwv", bufs=1) as pwv,
            tc.tile_pool(name="pw", bufs=6) as pw,
            tc.tile_pool(name="pat", bufs=1) as pat,
            tc.tile_pool(name="ppt", bufs=2) as ppt,
            tc.tile_pool(name="psm", bufs=2) as psm,
            tc.tile_pool(name="pout", bufs=2) as pout,
            tc.tile_pool(name="pssc", bufs=4, space="PSUM") as pssc,
            tc.tile_pool(name="pstl", bufs=2, space="PSUM") as pstl,
            tc.tile_pool(name="ppst", bufs=2, space="PSUM") as ppst,
        ):
            # ---- constants ----
            bo_sb = pc.tile([128, 8], F32, tag="bo")
            nc.sync.dma_start(bo_sb[:], bo.rearrange("(t p) -> p t", p=128))
            ml_sb = pc.tile([128, 1], BF16, tag="ml")
            mr_sb = pc.tile([128, 1], BF16, tag="mr")
            nc.sync.dma_start(ml_sb[:], maskl)
            nc.sync.dma_start(mr_sb[:], maskr)
            id_sb = pc.tile([128, 128], BF16, tag="ident")
            nc.sync.dma_start(id_sb[:], ident)
            gst = pc.tile([65, 32], F32, tag="gst")

            # ---- x slice, transposed, resident ----
            xts = []
            for d in range(8):
                xd = px.tile([128, TOKS], F32R, tag=f"xt{d}")
                rows = xt[128 * d:128 * (d + 1), :].bitcast(F32R)
                nc.sync.dma_start(xd[:, 0:512], rows[:, 0:512])
                nc.sync.dma_start(xd[:, 512:TOKS], rows[:, 512:TOKS])
                xts.append(xd)

            at_sb = [pat.tile([128, 1024], BF16, tag=f"at{f}", name=f"at{f}")
                     for f in range(8)]

            def proj_psum(i2, c, cn):
                # 6 simultaneous 1-bank accumulators: 4 from pssc, 2 from pstl
                pool = pssc if 3 * i2 + c < 4 else pstl
                return pool.tile([128, cn], F32, tag="ps1" if pool is pssc
                                 else "pstl", name=f"pj{i2}_{c}")

            def emit_qk_proj(pss, half, qk_tiles):
                for pname, wdram in (("q", wq), ("k", wk)):
                    osbs, psjs = [], []
                    for i2 in range(2):
                        i = 2 * half + i2
                        osb = pqk.tile([128, TOKS], BF16, tag=f"qk{pname}{i}",
                                       name=f"qk{pname}{i}")
                        qk_tiles[(pname, i)] = osb
                        osbs.append(osb)
                        psjs.append([proj_psum(i2, c, cn)
                                     for c, (c0, cn) in enumerate(CHUNKS)])
                    for d in range(8):
                        wt = pw.tile([128, 256], F32R, tag="w")
                        ft0 = 4 * pss + 2 * half
                        src = bass.AP(wdram.tensor,
                                      wdram[ft0, d].offset,
                                      [[128, 128], [8 * 128 * 128, 2], [1, 128]])
                        nc.sync.dma_start(wt[:], src.bitcast(F32R))
                        for i2 in range(2):
                            for c, (c0, cn) in enumerate(CHUNKS):
                                nc.tensor.matmul(
                                    psjs[i2][c][:, :cn],
                                    wt[:, 128 * i2:128 * i2 + 128],
                                    xts[d][:, c0:c0 + cn],
                                    start=(d == 0), stop=(d == 7))
                    for i2 in range(2):
                        for c, (c0, cn) in enumerate(CHUNKS):
                            if pname == "q":
                                nc.scalar.mul(osbs[i2][:, c0:c0 + cn],
                                              psjs[i2][c][:, :cn], SCALE)
                            else:
                                nc.scalar.copy(osbs[i2][:, c0:c0 + cn],
                                               psjs[i2][c][:, :cn])

            def emit_v_proj(pss):
                wv_sb = []
                for d in range(8):
                    wvd = pwv.tile([128, 512], F32R, tag=f"wv{d}", name=f"wv{d}")
                    nc.sync.dma_start(wvd[:], wv[pss, d].bitcast(F32R))
                    wv_sb.append(wvd)
                v96 = []
                for tb in range(NW):
                    pv_ps = pssc.tile([128, 512], F32, tag="ps1", name="pv_ps")
                    for d in range(8):
                        nc.tensor.matmul(pv_ps[:], xts[d][:, C(tb):C(tb) + 128],
                                         wv_sb[d][:], start=(d == 0), stop=(d == 7))
                    vt = pvp.tile([128, 8 * 96], BF16, tag=f"v96_{tb}",
                                  name=f"v96_{tb}")
                    pstep = vt.ap[0][0]
                    dst = bass.AP(vt.tensor, vt[:].offset,
                                  [[pstep, 128], [96, 8], [1, 64]])
                    src = bass.AP(pv_ps.tensor, pv_ps[:].offset,
                                  [[pv_ps.ap[0][0], 128], [64, 8], [1, 64]])
                    nc.vector.tensor_copy(dst, src)
                    onesap = bass.AP(vt.tensor, vt[:].offset + 64,
                                     [[pstep, 128], [96, 8], [1, 1]])
                    nc.vector.memset(onesap, 1.0)
                    v96.append(vt)
                pvg = pssc.tile([2, 512], F32, tag="ps1", name="pvg")
                for d in range(8):
                    nc.tensor.matmul(pvg[:], xts[d][:, 0:2], wv_sb[d][:],
                                     start=(d == 0), stop=(d == 7))
                vg_sb = pvp.tile([2, 8 * 96], BF16, tag="vg", name="vg")
                gstep = vg_sb.ap[0][0]
                gdst = bass.AP(vg_sb.tensor, vg_sb[:].offset,
                               [[gstep, 2], [96, 8], [1, 64]])
                gsrc = bass.AP(pvg.tensor, pvg[:].offset,
                               [[pvg.ap[0][0], 2], [64, 8], [1, 64]])
                nc.scalar.copy(gdst, gsrc)
                g1 = bass.AP(vg_sb.tensor, vg_sb[:].offset + 64,
                             [[gstep, 2], [96, 8], [1, 1]])
                nc.vector.memset(g1, 1.0)
                return v96, vg_sb

            def emit_scores(h, qk_tiles):
                """Scores + exps for head h: pt [kt, q], pxg [2, 1024], pg."""
                hl = h % 8
                r0 = 64 * (hl % 2)
                qh = qk_tiles[("q", hl // 2)][r0:r0 + 64, :]
                kh = qk_tiles[("k", hl // 2)][r0:r0 + 64, :]

                # local-q -> global-k scores, exp'd (separate softmax)
                pxg = psm.tile([2, 1024], BF16, tag="pxg", name="pxg", bufs=3)
                for c in range(2):
                    ps_xg = pssc.tile([2, 512], F32, tag="ps1", name="ps_xg")
                    nc.tensor.matmul(ps_xg[:], kh[:, 0:2],
                                     qh[:, LQ0 + 512 * c:LQ0 + 512 * c + 512],
                                     start=True, stop=True)
                    nc.scalar.activation(pxg[:, 512 * c:512 * c + 512], ps_xg[:],
                                         AF.Exp)
                # global-q -> local-k scores, exp'd (flash partials)
                psg = pssc.tile([128, 16], F32, tag="ps1", name="psg")
                for t in range(1, 9):
                    nc.tensor.matmul(psg[:, 2 * (t - 1):2 * t],
                                     kh[:, C(t):C(t) + 128], qh[:, 0:2],
                                     start=(t == 1), stop=(t == 8))
                pg = psm.tile([128, 16], BF16, tag="pgsb", name="pg", bufs=3)
                nc.scalar.activation(pg[:], psg[:], AF.Exp)

                # window scores [kt, q]; edge blocks share a psum bank so the
                # exp runs as one ACT instruction
                pt = ppt.tile([128, PT_COLS], BF16, tag="pt", name="pt")
                ps_e = pssc.tile([128, 384], F32, tag="ps1", name="ps_e")
                nc.tensor.matmul(ps_e[:, 0:128], kh[:, C(0):C(0) + 128],
                                 qh[:, C(1):C(1) + 128], start=True, stop=True)
                nc.tensor.matmul(ps_e[:, 128:384], kh[:, C(1):C(1) + 128],
                                 qh[:, C(1):C(1) + 256], start=True, stop=True)
                nc.scalar.activation(pt[:, 0:384], ps_e[:], AF.Exp)
                for t in range(2, 8):
                    ps_s = pssc.tile([128, 384], F32, tag="ps1", name="ps_s")
                    nc.tensor.matmul(ps_s[:], kh[:, C(t):C(t) + 128],
                                     qh[:, C(t - 1):C(t - 1) + 384],
                                     start=True, stop=True)
                    nc.scalar.activation(pt[:, PT_START[t]:PT_START[t] + 384],
                                         ps_s[:], AF.Exp)
                ps_f = pssc.tile([128, 384], F32, tag="ps1", name="ps_f")
                nc.tensor.matmul(ps_f[:, 0:256], kh[:, C(8):C(8) + 128],
                                 qh[:, C(7):C(7) + 256], start=True, stop=True)
                nc.tensor.matmul(ps_f[:, 256:384], kh[:, C(9):C(9) + 128],
                                 qh[:, C(8):C(8) + 128], start=True, stop=True)
                nc.scalar.activation(pt[:, 2688:3072], ps_f[:], AF.Exp)
                # edge masks: first/last local block of the batch row
                nc.gpsimd.tensor_mul(pt[:, 0:128], pt[:, 0:128],
                                     ml_sb[:].to_broadcast((128, 128)))
                nc.gpsimd.tensor_mul(pt[:, 2944:3072], pt[:, 2944:3072],
                                     mr_sb[:].to_broadcast((128, 128)))
                return {"pt": pt, "pxg": pxg, "pg": pg, "qh": qh, "kh": kh}

            def emit_tail(h, S, v96, vg_sb, psT_pair):
                """Flipped P.V + normalize + transpose for head h."""
                hl = h % 8
                r0 = 64 * (hl % 2)
                pt, pxg, pg = S["pt"], S["pxg"], S["pg"]
                for c in range(2):
                    psA = pstl.tile([128, 260], F32, tag="pstl", name="psA")
                    psB = pstl.tile([128, 260], F32, tag="pstl", name="psB")
                    for j in range(4):
                        qs = 4 * c + j + 1
                        win = (qs - 1, qs, qs + 1)
                        for i, t in enumerate(win):
                            nc.tensor.matmul(
                                psA[:, 65 * j:65 * j + 65],
                                pt[:, ptcol(t, qs):ptcol(t, qs) + 128],
                                v96[t][:, 96 * hl:96 * hl + 65],
                                start=(i == 0), stop=(i == 2))
                        nc.tensor.matmul(
                            psB[:, 65 * j:65 * j + 65],
                            pxg[:, 128 * (qs - 1):128 * qs],
                            vg_sb[:, 96 * hl:96 * hl + 65],
                            start=True, stop=True)
                    pA = psA.ap[0][0]
                    pB = psB.ap[0][0]
                    rA = psm.tile([128, 4], F32, tag="rA", name="rA")
                    rB = psm.tile([128, 4], F32, tag="rB", name="rB")
                    nc.vector.reciprocal(
                        rA[:], bass.AP(psA.tensor, psA[:].offset + 64,
                                       [[pA, 128], [65, 4]]))
                    nc.vector.reciprocal(
                        rB[:], bass.AP(psB.tensor, psB[:].offset + 64,
                                       [[pB, 128], [65, 4]]))
                    numA = bass.AP(psA.tensor, psA[:].offset,
                                   [[pA, 128], [65, 4], [1, 64]])
                    numB = bass.AP(psB.tensor, psB[:].offset,
                                   [[pB, 128], [65, 4], [1, 64]])
                    rAb = bass.AP(rA.tensor, rA[:].offset,
                                  [[rA.ap[0][0], 128], [1, 4], [0, 64]])
                    rBb = bass.AP(rB.tensor, rB[:].offset,
                                  [[rB.ap[0][0], 128], [1, 4], [0, 64]])
                    tmpA = psm.tile([128, 256], F32, tag="tmpA", name="tmpA")
                    tmpB = psm.tile([128, 256], F32, tag="tmpB", name="tmpB")
                    tA = bass.AP(tmpA.tensor, tmpA[:].offset,
                                 [[tmpA.ap[0][0], 128], [64, 4], [1, 64]])
                    tB = bass.AP(tmpB.tensor, tmpB[:].offset,
                                 [[tmpB.ap[0][0], 128], [64, 4], [1, 64]])
                    nc.vector.tensor_mul(tA, numA, rAb)
                    nc.vector.tensor_mul(tB, numB, rBb)
                    atq = psm.tile([128, 256], BF16, tag="atq", name="atq")
                    nc.gpsimd.tensor_add(atq[:], tmpA[:], tmpB[:])
                    for j in range(4):
                        nc.tensor.transpose(
                            psT_pair[c][r0:r0 + 64, 128 * j:128 * j + 128],
                            atq[:, 64 * j:64 * j + 64], id_sb[:])
                # flash partials of the 2 global queries vs this core's keys
                ps_wv = pstl.tile([65, 2], F32, tag="pstl", name="ps_wv")
                for t in range(1, 9):
                    nc.tensor.matmul(ps_wv[:], v96[t][:, 96 * hl:96 * hl + 65],
                                     pg[:, 2 * (t - 1):2 * t],
                                     start=(t == 1), stop=(t == 8))
                nc.scalar.copy(gst[:, 2 * h:2 * h + 2], ps_wv[:])
                if hl % 2 == 1:
                    for c in range(2):
                        nc.vector.tensor_copy(
                            at_sb[h // 2][:, 512 * c:512 * c + 512],
                            psT_pair[c][:])

            # software-pipelined emission: scores of head h+1 are emitted
            # before the tail of head h; pass-B projections interleave at
            # group boundaries (PE executes in program order)
            qk0, qk1 = {}, {}
            emit_qk_proj(0, 0, qk0)
            emit_qk_proj(0, 1, qk0)
            v96_0, vg0 = emit_v_proj(0)
            v96_1, vg1 = None, None
            S = {0: emit_scores(0, qk0)}
            psT_pair = None
            for h in range(16):
                if h + 1 < 16:
                    S[h + 1] = emit_scores(h + 1, qk0 if h + 1 < 8 else qk1)
                if h % 2 == 0:
                    psT_pair = [ppst.tile([128, 512], BF16, tag="psT",
                                          name=f"psT{h}_{c}")
                                for c in range(2)]
                v96, vg = (v96_0, vg0) if h < 8 else (v96_1, vg1)
                emit_tail(h, S.pop(h), v96, vg, psT_pair)
                if h == 2:
                    emit_qk_proj(1, 0, qk1)
                if h == 6:
                    emit_qk_proj(1, 1, qk1)
                if h == 7:
                    v96_1, vg1 = emit_v_proj(1)

            # ================= output projection =================
            # prefetch the first weight tiles before the barrier so their DMAs
            # land during the attention tail
            wot_pre = []
            for m in range(2):
                wotp = pw.tile([128, 1024], BF16, tag="wo", bufs=3,
                               name=f"wot{m}")
                wsrc = bass.AP(wo.tensor, wo[m, 0].offset,
                               [[128, 128], [128 * 128, 8], [1, 128]])
                nc.sync.dma_start(wotp[:], wsrc)
                wot_pre.append(wotp)
            tc.no_sync_barrier()
            for m in range(8):
                ps_op = [pssc.tile([128, 512], F32, tag="ps1", name=f"pop{c}")
                         for c in range(2)]
                if m < 2:
                    wot = wot_pre[m]
                else:
                    wot = pw.tile([128, 1024], BF16, tag="wo", bufs=3)
                    wsrc = bass.AP(wo.tensor, wo[m, 0].offset,
                                   [[128, 128], [128 * 128, 8], [1, 128]])
                    nc.sync.dma_start(wot[:], wsrc)
                for f in range(8):
                    for c in range(2):
                        nc.tensor.matmul(ps_op[c][:], wot[:, 128 * f:128 * f + 128],
                                         at_sb[f][:, 512 * c:512 * c + 512],
                                         start=(f == 0), stop=(f == 7))
                for c in range(2):
                    ot = pout.tile([128, 512], F32, tag="ot")
                    nc.scalar.activation(ot[:], ps_op[c][:], AF.Identity,
                                         bias=bo_sb[:, m:m + 1])
                    nc.sync.dma_start(outt[128 * m:128 * (m + 1),
                                           512 * c:512 * c + 512], ot[:])
            nc.sync.dma_start(gstats, gst[:])
            if dbg is not None:
                for f in range(8):
                    nc.sync.dma_start(dbg[128 * f:128 * (f + 1), :], at_sb[f][:])
    return nc


_NC_CACHE = {}
LAST = {}


def get_nc():
    if "nc" not in _NC_CACHE:
        nc = bacc.Bacc("TRN2", target_bir_lowering=False, debug=False, num_devices=8)
        build_kernel(nc)
        nc.compile()
        _NC_CACHE["nc"] = nc
    return _NC_CACHE["nc"]


def make_inputs(x, Wq, Wk, Wv, Wo, bo):
    """Build the 8 per-core input maps (all host-side numpy)."""
    x = np.asarray(x, np.float32)
    Wq = np.asarray(Wq, np.float32)
    Wk = np.asarray(Wk, np.float32)
    Wv = np.asarray(Wv, np.float32)
    Wo = np.asarray(Wo, np.float32)
    bo = np.asarray(bo, np.float32)

    wq_r = np.ascontiguousarray(
        Wq.T.reshape(8, 128, 8, 128).transpose(2, 0, 1, 3))  # [ft, d, 128d, 128f]
    wk_r = np.ascontiguousarray(Wk.T.reshape(8, 128, 8, 128).transpose(2, 0, 1, 3))
    wv_r = np.ascontiguousarray(
        Wv.T.reshape(8, 128, 2, 512).transpose(2, 0, 1, 3))  # [fh, d, 128d, 512f]
    wo_r = np.ascontiguousarray(
        Wo.T.reshape(8, 128, 8, 128).transpose(2, 0, 1, 3)).astype(BF)
    # wo_r[m, f, i, j] must be Wo[128m+j, 128f+i] = Wo.T[128f+i, 128m+j]

    ones = np.ones((128, 1), BF)
    zeros = np.zeros((128, 1), BF)
    ident = np.eye(128, dtype=BF)
    in_maps = []
    for core in range(8):
        b, j = divmod(core, 4)
        xs = np.zeros((TOKS, D_MODEL), np.float32)
        xs[0] = x[b, 0]
        xs[1] = x[b, T - 1]
        for w in range(NW):
            gb = 8 * j - 1 + w
            if 0 <= gb < NB:
                xs[2 + 128 * w:2 + 128 * (w + 1)] = x[b, 1 + 128 * gb:1 + 128 * (gb + 1)]
        in_maps.append({
            "xt": np.ascontiguousarray(xs.T),
            "wq": wq_r, "wk": wk_r, "wv": wv_r, "wo": wo_r, "bo": bo,
            "maskl": zeros if j == 0 else ones,
            "maskr": zeros if j == 3 else ones,
            "ident": ident,
        })
    return in_maps


def assemble_output(results, x, Wq, Wk, Wv, Wo, bo):
    x = np.asarray(x, np.float32)
    out = np.empty((B, T, D_MODEL), np.float32)
    for core in range(8):
        b, j = divmod(core, 4)
        out[b, 1 + 1024 * j:1 + 1024 * (j + 1), :] = results[core]["outt"].T

    # global token rows, exact on host
    xg = x[:, [0, T - 1], :]                      # [B, 2, D]
    qg = (xg @ Wq.T).reshape(B, 2, H, DK) * SCALE  # [B, 2, H, DK]
    kg = (xg @ Wk.T).reshape(B, 2, H, DK)
    vg = (xg @ Wv.T).reshape(B, 2, H, DK)
    for b in range(B):
        se = np.zeros((H, 2))
        wvs = np.zeros((H, 2, DK))
        for j in range(4):
            g = results[4 * b + j]["gstats"]  # [65, 32]
            for h in range(H):
                for gi in range(2):
                    se[h, gi] += g[64, 2 * h + gi]
                    wvs[h, gi] += g[0:64, 2 * h + gi]
        # add the global-key terms: scores qg . kg
        sgg = np.einsum("ghd,fhd->hgf", qg[b], kg[b])  # [H, 2g(query), 2f(key)]
        egg = np.exp(sgg)
        num = wvs + np.einsum("hgf,fhd->hgd", egg, vg[b])
        den = se + egg.sum(-1)
        og = num / den[..., None]                  # [H, 2, DK]
        for gi, trow in ((0, 0), (1, T - 1)):
            row = og[:, gi, :].reshape(H * DK)
            out[b, trow] = row @ Wo.T + bo
    return out


def kernel(x, Wq, Wk, Wv, Wo, bo):
    nc = get_nc()
    in_maps = make_inputs(x, Wq, Wk, Wv, Wo, bo)
    res = run_bass_kernel_spmd(nc, in_maps, core_ids=list(range(8)))
    LAST["res"] = res
    results = [{k: np.asarray(v) for k, v in r.items()} for r in res.results]
    return assemble_output(results, x, Wq, Wk, Wv, Wo, bo)


# revision 15
# speedup vs baseline: 1.7815x; 1.0271x over previous
"""BigBird sparse attention kernel for 8 Trainium2 NeuronCores.

Sharding: token-parallel. B=2 batches x 4 chunks of 1024 local tokens each
-> 8 cores. Each core receives a transposed x-slice [D=1024, 1282] whose
columns are [g0, g1, 10 window blocks of 128 tokens] (blocks 8j-1 .. 8j+8,
zero-padded outside [0, 32)). The core computes:
  - q/k projections in transposed layout [f, tok] (fp32r matmuls, bf16 out)
  - v projection in [tok, f] layout with a ones-column per head
  - 3-block sliding-window attention: scores kept transposed [kt, q], exp'd
    to bf16 probabilities; the P.V matmul is FLIPPED (stationary = P block,
    moving = V||ones) so each 128-token q block lands in PSUM as
    [q, 64 v-cols + denominator] with the softmax denominator per-partition
  - attention of local tokens to the 2 global tokens (separate softmax,
    same flipped layout) -> normalize both with per-partition reciprocals,
    combine, transpose back to [feat, tok] on the PE array
  - flash-style partial stats (sum-exp, weighted V) of the 2 global query
    tokens against the core's local keys -> combined on host
  - output projection + bias for its 1024 local tokens
Host assembles the 8 slices, and computes the 2 global output rows per
batch exactly in numpy from the shipped partials.
"""

import numpy as np
import ml_dtypes

import concourse.bass as bass
import concourse.mybir as mybir
import concourse.tile as tile
from concourse import bacc
from concourse.bass_utils import run_bass_kernel_spmd

F32 = mybir.dt.float32
F32R = mybir.dt.float32r
BF16 = mybir.dt.bfloat16
AF = mybir.ActivationFunctionType
BF = ml_dtypes.bfloat16

D_MODEL = 1024
H = 16
DK = 64
BS = 128
B = 2
T = 4098
NB = 32            # global 128-blocks of local tokens
NW = 10            # window blocks per core (8 local + 2 halo)
TOKS = 2 + NW * BS # x-slice columns
SCALE = 1.0 / np.sqrt(DK)

# x-slice column layout: [g0, g1, L0..L7 (8*128 local), HL (128), HR (128)]
# q is only needed for globals + locals (cols 0:1026), k/v for everything.
CHUNKS_K = [(0, 512), (512, 512), (1024, 258)]
CHUNKS_Q = [(0, 512), (512, 512), (1024, 2)]

# pt region start per window block t (regions sized by the q-window width)
PT_START = [0, 128, 384, 768, 1152, 1536, 1920, 2304, 2688, 2944]
PT_COLS = 3072


def CK(t):
    # column of window block t in the x-slice (k/v side)
    return 1026 if t == 0 else (1154 if t == 9 else 2 + BS * (t - 1))


def CQ(qs):
    # column of local q block qs (1..8) in the x-slice
    return 2 + BS * (qs - 1)


def qlo(t):
    return max(t - 1, 1)


def ptcol(t, qs):
    # column of (window-block t, q window-position qs) in the pt tensor
    return PT_START[t] + 128 * (qs - qlo(t))


def build_kernel(nc):
    xt = nc.dram_tensor("xt", [D_MODEL, TOKS], F32, kind="ExternalInput").ap()
    wqk = nc.dram_tensor("wqk", [2, 2, 8, 128, 512], F32,
                         kind="ExternalInput").ap()
    wv = nc.dram_tensor("wv", [2, 8, 128, 512], F32, kind="ExternalInput").ap()
    wo = nc.dram_tensor("wo", [8, 8, 128, 128], BF16, kind="ExternalInput").ap()
    bo = nc.dram_tensor("bo", [D_MODEL], F32, kind="ExternalInput").ap()
    maskl = nc.dram_tensor("maskl", [128, 1], BF16, kind="ExternalInput").ap()
    maskr = nc.dram_tensor("maskr", [128, 1], BF16, kind="ExternalInput").ap()
    ident = nc.dram_tensor("ident", [128, 128], BF16, kind="ExternalInput").ap()
    outt = nc.dram_tensor("outt", [D_MODEL, 1024], F32, kind="ExternalOutput").ap()
    gstats = nc.dram_tensor("gstats", [65, 32], F32, kind="ExternalOutput").ap()
    import os as _os
    dbg = None
    if _os.environ.get("BB_DEBUG"):
        dbg = nc.dram_tensor("dbg_at", [D_MODEL, 1024], BF16,
                             kind="ExternalOutput").ap()

    with tile.TileContext(nc) as tc:
        with (
            tc.tile_pool(name="pc", bufs=1) as pc,
            tc.tile_pool(name="px", bufs=1) as px,
            tc.tile_pool(name="pqk", bufs=1) as pqk,
            tc.tile_pool(name="pv", bufs=1) as pvp,
            tc.tile_pool(name="pwv", bufs=1) as pwv,
            tc.tile_pool(name="pw", bufs=6) as pw,
            tc.tile_pool(name="pat", bufs=1) as pat,
            tc.tile_pool(name="ppt", bufs=2) as ppt,
            tc.tile_pool(name="psm", bufs=2) as psm,
            tc.tile_pool(name="pout", bufs=2) as pout,
            tc.tile_pool(name="pssc", bufs=4, space="PSUM") as pssc,
            tc.tile_pool(name="pstl", bufs=2, space="PSUM") as pstl,
            tc.tile_pool(name="ppst", bufs=2, space="PSUM") as ppst,
        ):
            # ---- constants ----
            bo_sb = pc.tile([128, 8], F32, tag="bo")
            nc.sync.dma_start(bo_sb[:], bo.rearrange("(t p) -> p t", p=128))
            ml_sb = pc.tile([128, 1], BF16, tag="ml")
            mr_sb = pc.tile([128, 1], BF16, tag="mr")
            nc.sync.dma_start(ml_sb[:], maskl)
            nc.sync.dma_start(mr_sb[:], maskr)
            id_sb = pc.tile([128, 128], BF16, tag="ident")
            nc.sync.dma_start(id_sb[:], ident)
            gst = pc.tile([65, 32], F32, tag="gst")

            def wqk_dma(pss, half, d, name):
                # one [128, 512] tile: q feature-tiles (2) then k feature-tiles
                wt = pw.tile([128, 512], F32R, tag="w", bufs=8, name=name)
                nc.sync.dma_start(wt[:], wqk[pss, half, d].bitcast(F32R))
                return wt

            # ---- x slice, transposed, resident; first-pass q/k weights
            # interleave so the projection d-loop starts immediately ----
            xts, wts0 = [], []
            for d in range(8):
                wts0.append(wqk_dma(0, 0, d, f"wt0_{d}"))
                xd = px.tile([128, TOKS], F32R, tag=f"xt{d}")
                rows = xt[128 * d:128 * (d + 1), :].bitcast(F32R)
                nc.sync.dma_start(xd[:, 0:512], rows[:, 0:512])
                nc.sync.dma_start(xd[:, 512:TOKS], rows[:, 512:TOKS])
                xts.append(xd)

            at_sb = [pat.tile([128, 1024], BF16, tag=f"at{f}", name=f"at{f}")
                     for f in range(8)]

            def proj_psum(i2, c, cn):
                # 6 simultaneous 1-bank accumulators: 4 from pssc, 2 from pstl
                pool = pssc if 3 * i2 + c < 4 else pstl
                return pool.tile([128, cn], F32, tag="ps1" if pool is pssc
                                 else "pstl", name=f"pj{i2}_{c}")

            def emit_qk_proj(pss, half, qk_tiles, pre=None):
                wts = pre if pre is not None else [
                    wqk_dma(pss, half, d, f"wt{pss}{half}_{d}")
                    for d in range(8)]
                for pi, pname in enumerate(("q", "k")):
                    chunks = CHUNKS_Q if pname == "q" else CHUNKS_K
                    osbs, psjs = [], []
                    for i2 in range(2):
                        i = 2 * half + i2
                        osb = pqk.tile([128, TOKS], BF16, tag=f"qk{pname}{i}",
                                       name=f"qk{pname}{i}")
                        qk_tiles[(pname, i)] = osb
                        osbs.append(osb)
                        psjs.append([proj_psum(i2, c, cn)
                                     for c, (c0, cn) in enumerate(chunks)])
                    for d in range(8):
                        for i2 in range(2):
                            for c, (c0, cn) in enumerate(chunks):
                                nc.tensor.matmul(
                                    psjs[i2][c][:, :cn],
                                    wts[d][:, 256 * pi + 128 * i2:
                                           256 * pi + 128 * i2 + 128],
                                    xts[d][:, c0:c0 + cn],
                                    start=(d == 0), stop=(d == 7))
                    for i2 in range(2):
                        for c, (c0, cn) in enumerate(chunks):
                            if pname == "q":
                                nc.scalar.mul(osbs[i2][:, c0:c0 + cn],
                                              psjs[i2][c][:, :cn], SCALE)
                            else:
                                nc.scalar.copy(osbs[i2][:, c0:c0 + cn],
                                               psjs[i2][c][:, :cn])

            def emit_v_proj(pss):
                wv_sb = []
                for d in range(8):
                    wvd = pwv.tile([128, 512], F32R, tag=f"wv{d}", name=f"wv{d}")
                    nc.sync.dma_start(wvd[:], wv[pss, d].bitcast(F32R))
                    wv_sb.append(wvd)
                v96 = []
                for tb in range(NW):
                    pv_ps = pssc.tile([128, 512], F32, tag="ps1", name="pv_ps")
                    for d in range(8):
                        nc.tensor.matmul(pv_ps[:], xts[d][:, CK(tb):CK(tb) + 128],
                                         wv_sb[d][:], start=(d == 0), stop=(d == 7))
                    vt = pvp.tile([128, 8 * 96], BF16, tag=f"v96_{tb}",
                                  name=f"v96_{tb}")
                    pstep = vt.ap[0][0]
                    dst = bass.AP(vt.tensor, vt[:].offset,
                                  [[pstep, 128], [96, 8], [1, 64]])
                    src = bass.AP(pv_ps.tensor, pv_ps[:].offset,
                                  [[pv_ps.ap[0][0], 128], [64, 8], [1, 64]])
                    nc.vector.tensor_copy(dst, src)
                    onesap = bass.AP(vt.tensor, vt[:].offset + 64,
                                     [[pstep, 128], [96, 8], [1, 1]])
                    nc.vector.memset(onesap, 1.0)
                    v96.append(vt)
                pvg = pssc.tile([2, 512], F32, tag="ps1", name="pvg")
                for d in range(8):
                    nc.tensor.matmul(pvg[:], xts[d][:, 0:2], wv_sb[d][:],
                                     start=(d == 0), stop=(d == 7))
                # vg lives twice: partitions 0:2 (even heads) and 32:34 (odd
                # heads, whose paired-pxg stationary sits at partition 32)
                vg_sb = pvp.tile([34, 8 * 96], BF16, tag="vg", name="vg")
                gstep = vg_sb.ap[0][0]
                gsrc = bass.AP(pvg.tensor, pvg[:].offset,
                               [[pvg.ap[0][0], 2], [64, 8], [1, 64]])
                for p0 in (0, 32):
                    gdst = bass.AP(vg_sb.tensor, vg_sb[p0:p0 + 2, :].offset,
                                   [[gstep, 2], [96, 8], [1, 64]])
                    nc.scalar.copy(gdst, gsrc)
                    g1 = bass.AP(vg_sb.tensor, vg_sb[p0:p0 + 2, :].offset + 64,
                                 [[gstep, 2], [96, 8], [1, 1]])
                    nc.vector.memset(g1, 1.0)
                return v96, vg_sb

            pxg_pairs = {}

            def emit_scores(h, qk_tiles):
                """Scores + exps for head h: pt [kt, q], paired pxg, pg."""
                hl = h % 8
                r0 = 64 * (hl % 2)
                qt = qk_tiles[("q", hl // 2)]
                kt_ = qk_tiles[("k", hl // 2)]
                qh = qt[r0:r0 + 64, :]
                kh = kt_[r0:r0 + 64, :]

                if h % 2 == 0:
                    # local-q -> global-k scores for BOTH heads of the pair in
                    # one matmul per 512-token chunk: stationary [128, 34] with
                    # head-even kg in (rows 0:64, cols 0:2) and head-odd kg in
                    # (rows 64:128, cols 32:34); zeros elsewhere mask the
                    # cross-head terms.
                    kg2 = psm.tile([128, 34], BF16, tag="kg2", name="kg2")
                    nc.vector.memset(kg2[:], 0.0)
                    nc.vector.tensor_copy(kg2[0:64, 0:2], kt_[0:64, 0:2])
                    nc.vector.tensor_copy(kg2[64:128, 32:34], kt_[64:128, 0:2])
                    pxg2 = psm.tile([34, 1024], BF16, tag="pxg", name="pxg2",
                                    bufs=2)
                    for c in range(2):
                        ps_xg = pssc.tile([34, 512], F32, tag="ps1",
                                          name="ps_xg")
                        nc.tensor.matmul(ps_xg[:], kg2[:],
                                         qt[:, 2 + 512 * c:2 + 512 * c + 512],
                                         start=True, stop=True)
                        nc.scalar.activation(pxg2[:, 512 * c:512 * c + 512],
                                             ps_xg[:], AF.Exp)
                    pxg_pairs[h // 2] = pxg2
                pxg2 = pxg_pairs[h // 2]

                # window scores [kt, q] with the global-q scores (psg) riding
                # each k-block's stationary; edge blocks share a psum bank so
                # the exp runs as one ACT instruction
                psg = pssc.tile([128, 16], F32, tag="ps1", name="psg")
                pt = ppt.tile([128, PT_COLS], BF16, tag="pt", name="pt")
                ps_e = pssc.tile([128, 384], F32, tag="ps1", name="ps_e")
                nc.tensor.matmul(ps_e[:, 0:128], kh[:, CK(0):CK(0) + 128],
                                 qh[:, CQ(1):CQ(1) + 128], start=True, stop=True)
                nc.tensor.matmul(ps_e[:, 128:384], kh[:, CK(1):CK(1) + 128],
                                 qh[:, CQ(1):CQ(1) + 256], start=True, stop=True)
                nc.tensor.matmul(psg[:, 0:2], kh[:, CK(1):CK(1) + 128],
                                 qh[:, 0:2], start=True, stop=False)
                nc.scalar.activation(pt[:, 0:384], ps_e[:], AF.Exp)
                for t in range(2, 8):
                    ps_s = pssc.tile([128, 384], F32, tag="ps1", name="ps_s")
                    nc.tensor.matmul(ps_s[:], kh[:, CK(t):CK(t) + 128],
                                     qh[:, CQ(t - 1):CQ(t - 1) + 384],
                                     start=True, stop=True)
                    nc.tensor.matmul(psg[:, 2 * (t - 1):2 * t],
                                     kh[:, CK(t):CK(t) + 128], qh[:, 0:2],
                                     start=False, stop=False)
                    nc.scalar.activation(pt[:, PT_START[t]:PT_START[t] + 384],
                                         ps_s[:], AF.Exp)
                ps_f = pssc.tile([128, 384], F32, tag="ps1", name="ps_f")
                nc.tensor.matmul(ps_f[:, 0:256], kh[:, CK(8):CK(8) + 128],
                                 qh[:, CQ(7):CQ(7) + 256], start=True, stop=True)
                nc.tensor.matmul(psg[:, 14:16], kh[:, CK(8):CK(8) + 128],
                                 qh[:, 0:2], start=False, stop=True)
                nc.tensor.matmul(ps_f[:, 256:384], kh[:, CK(9):CK(9) + 128],
                                 qh[:, CQ(8):CQ(8) + 128], start=True, stop=True)
                pg = psm.tile([128, 16], BF16, tag="pgsb", name="pg", bufs=3)
                nc.scalar.activation(pg[:], psg[:], AF.Exp)
                nc.scalar.activation(pt[:, 2688:3072], ps_f[:], AF.Exp)
                # edge masks: first/last local block of the batch row
                nc.gpsimd.tensor_mul(pt[:, 0:128], pt[:, 0:128],
                                     ml_sb[:].to_broadcast((128, 128)))
                nc.gpsimd.tensor_mul(pt[:, 2944:3072], pt[:, 2944:3072],
                                     mr_sb[:].to_broadcast((128, 128)))
                return {"pt": pt, "pxg2": pxg2, "pg": pg}

            def emit_tail(h, S, v96, vg_sb, psT_pair):
                """Flipped P.V + normalize + transpose for head h."""
                hl = h % 8
                r0 = 64 * (hl % 2)
                r2 = 32 * (h % 2)
                pt, pxg2, pg = S["pt"], S["pxg2"], S["pg"]
                for c in range(2):
                    psA = pstl.tile([128, 260], F32, tag="pstl", name="psA")
                    psB = pstl.tile([128, 260], F32, tag="pstl", name="psB")
                    for j in range(4):
                        qs = 4 * c + j + 1
                        win = (qs - 1, qs, qs + 1)
                        for i, t in enumerate(win):
                            nc.tensor.matmul(
                                psA[:, 65 * j:65 * j + 65],
                                pt[:, ptcol(t, qs):ptcol(t, qs) + 128],
                                v96[t][:, 96 * hl:96 * hl + 65],
                                start=(i == 0), stop=(i == 2))
                        nc.tensor.matmul(
                            psB[:, 65 * j:65 * j + 65],
                            pxg2[r2:r2 + 2, 128 * (qs - 1):128 * qs],
                            vg_sb[r2:r2 + 2, 96 * hl:96 * hl + 65],
                            start=True, stop=True)
                    pA = psA.ap[0][0]
                    pB = psB.ap[0][0]
                    rA = psm.tile([128, 4], F32, tag="rA", name="rA")
                    rB = psm.tile([128, 4], F32, tag="rB", name="rB")
                    nc.vector.reciprocal(
                        rA[:], bass.AP(psA.tensor, psA[:].offset + 64,
                                       [[pA, 128], [65, 4]]))
                    nc.vector.reciprocal(
                        rB[:], bass.AP(psB.tensor, psB[:].offset + 64,
                                       [[pB, 128], [65, 4]]))
                    numA = bass.AP(psA.tensor, psA[:].offset,
                                   [[pA, 128], [65, 4], [1, 64]])
                    numB = bass.AP(psB.tensor, psB[:].offset,
                                   [[pB, 128], [65, 4], [1, 64]])
                    rAb = bass.AP(rA.tensor, rA[:].offset,
                                  [[rA.ap[0][0], 128], [1, 4], [0, 64]])
                    rBb = bass.AP(rB.tensor, rB[:].offset,
                                  [[rB.ap[0][0], 128], [1, 4], [0, 64]])
                    tmpA = psm.tile([128, 256], F32, tag="tmpA", name="tmpA")
                    tmpB = psm.tile([128, 256], F32, tag="tmpB", name="tmpB")
                    tA = bass.AP(tmpA.tensor, tmpA[:].offset,
                                 [[tmpA.ap[0][0], 128], [64, 4], [1, 64]])
                    tB = bass.AP(tmpB.tensor, tmpB[:].offset,
                                 [[tmpB.ap[0][0], 128], [64, 4], [1, 64]])
                    nc.vector.tensor_mul(tA, numA, rAb)
                    nc.vector.tensor_mul(tB, numB, rBb)
                    atq = psm.tile([128, 256], BF16, tag="atq", name="atq")
                    nc.gpsimd.tensor_add(atq[:], tmpA[:], tmpB[:])
                    for j in range(4):
                        nc.tensor.transpose(
                            psT_pair[c][r0:r0 + 64, 128 * j:128 * j + 128],
                            atq[:, 64 * j:64 * j + 64], id_sb[:])
                # flash partials of the 2 global queries vs this core's keys
                ps_wv = pstl.tile([65, 2], F32, tag="pstl", name="ps_wv")
                for t in range(1, 9):
                    nc.tensor.matmul(ps_wv[:], v96[t][:, 96 * hl:96 * hl + 65],
                                     pg[:, 2 * (t - 1):2 * t],
                                     start=(t == 1), stop=(t == 8))
                nc.scalar.copy(gst[:, 2 * h:2 * h + 2], ps_wv[:])
                if hl % 2 == 1:
                    for c in range(2):
                        nc.vector.tensor_copy(
                            at_sb[h // 2][:, 512 * c:512 * c + 512],
                            psT_pair[c][:])

            # software-pipelined emission: scores of head h+1 are emitted
            # before the tail of head h; pass-B projections interleave at
            # group boundaries (PE executes in program order)
            qk0, qk1 = {}, {}
            emit_qk_proj(0, 0, qk0, pre=wts0)
            emit_qk_proj(0, 1, qk0)
            v96_0, vg0 = emit_v_proj(0)
            v96_1, vg1 = None, None
            S = {0: emit_scores(0, qk0)}
            psT_pair = None
            for h in range(16):
                if h + 1 < 16:
                    S[h + 1] = emit_scores(h + 1, qk0 if h + 1 < 8 else qk1)
                if h % 2 == 0:
                    psT_pair = [ppst.tile([128, 512], BF16, tag="psT",
                                          name=f"psT{h}_{c}")
                                for c in range(2)]
                v96, vg = (v96_0, vg0) if h < 8 else (v96_1, vg1)
                emit_tail(h, S.pop(h), v96, vg, psT_pair)
                if h == 2:
                    emit_qk_proj(1, 0, qk1)
                if h == 6:
                    emit_qk_proj(1, 1, qk1)
                if h == 7:
                    v96_1, vg1 = emit_v_proj(1)

            # ================= output projection =================
            # prefetch the first weight tiles before the barrier so their DMAs
            # land during the attention tail
            wot_pre = []
            for m in range(2):
                wotp = pw.tile([128, 1024], BF16, tag="wo", bufs=3,
                               name=f"wot{m}")
                wsrc = bass.AP(wo.tensor, wo[m, 0].offset,
                               [[128, 128], [128 * 128, 8], [1, 128]])
                nc.sync.dma_start(wotp[:], wsrc)
                wot_pre.append(wotp)
            tc.no_sync_barrier()
            for m in range(8):
                ps_op = [pssc.tile([128, 512], F32, tag="ps1", name=f"pop{c}")
                         for c in range(2)]
                if m < 2:
                    wot = wot_pre[m]
                else:
                    wot = pw.tile([128, 1024], BF16, tag="wo", bufs=3)
                    wsrc = bass.AP(wo.tensor, wo[m, 0].offset,
                                   [[128, 128], [128 * 128, 8], [1, 128]])
                    nc.sync.dma_start(wot[:], wsrc)
                for f in range(8):
                    for c in range(2):
                        nc.tensor.matmul(ps_op[c][:], wot[:, 128 * f:128 * f + 128],
                                         at_sb[f][:, 512 * c:512 * c + 512],
                                         start=(f == 0), stop=(f == 7))
                for c in range(2):
                    ot = pout.tile([128, 512], F32, tag="ot")
                    nc.scalar.activation(ot[:], ps_op[c][:], AF.Identity,
                                         bias=bo_sb[:, m:m + 1])
                    nc.sync.dma_start(outt[128 * m:128 * (m + 1),
                                           512 * c:512 * c + 512], ot[:])
            nc.sync.dma_start(gstats, gst[:])
            if dbg is not None:
                for f in range(8):
                    nc.sync.dma_start(dbg[128 * f:128 * (f + 1), :], at_sb[f][:])
    return nc


_NC_CACHE = {}
LAST = {}


def get_nc():
    if "nc" not in _NC_CACHE:
        nc = bacc.Bacc("TRN2", target_bir_lowering=False, debug=False, num_devices=8)
        build_kernel(nc)
        nc.compile()
        _NC_CACHE["nc"] = nc
    return _NC_CACHE["nc"]


def make_inputs(x, Wq, Wk, Wv, Wo, bo):
    """Build the 8 per-core input maps (all host-side numpy)."""
    x = np.asarray(x, np.float32)
    Wq = np.asarray(Wq, np.float32)
    Wk = np.asarray(Wk, np.float32)
    Wv = np.asarray(Wv, np.float32)
    Wo = np.asarray(Wo, np.float32)
    bo = np.asarray(bo, np.float32)

    wq_r = Wq.T.reshape(8, 128, 8, 128).transpose(2, 0, 1, 3)  # [ft, d, 128d, 128f]
    wk_r = Wk.T.reshape(8, 128, 8, 128).transpose(2, 0, 1, 3)
    # [pss, half, d, 128d, 512]: per (pass, half, d) the 512 cols are
    # [q ft0 | q ft1 | k ft0 | k ft1]
    qp = wq_r.reshape(2, 2, 2, 8, 128, 128).transpose(0, 1, 3, 4, 2, 5)
    kp = wk_r.reshape(2, 2, 2, 8, 128, 128).transpose(0, 1, 3, 4, 2, 5)
    wqk_r = np.ascontiguousarray(np.concatenate(
        [qp.reshape(2, 2, 8, 128, 256), kp.reshape(2, 2, 8, 128, 256)], -1))
    wv_r = np.ascontiguousarray(
        Wv.T.reshape(8, 128, 2, 512).transpose(2, 0, 1, 3))  # [fh, d, 128d, 512f]
    wo_r = np.ascontiguousarray(
        Wo.T.reshape(8, 128, 8, 128).transpose(2, 0, 1, 3)).astype(BF)
    # wo_r[m, f, i, j] must be Wo[128m+j, 128f+i] = Wo.T[128f+i, 128m+j]

    ones = np.ones((128, 1), BF)
    zeros = np.zeros((128, 1), BF)
    ident = np.eye(128, dtype=BF)
    in_maps = []
    for core in range(8):
        b, j = divmod(core, 4)
        # x-slice columns: [g0, g1, L0..L7, halo-left, halo-right]
        xs = np.zeros((TOKS, D_MODEL), np.float32)
        xs[0] = x[b, 0]
        xs[1] = x[b, T - 1]
        for w in range(NW):
            gb = 8 * j - 1 + w
            col = 1026 if w == 0 else (1154 if w == 9 else 2 + 128 * (w - 1))
            if 0 <= gb < NB:
                xs[col:col + 128] = x[b, 1 + 128 * gb:1 + 128 * (gb + 1)]
        in_maps.append({
            "xt": np.ascontiguousarray(xs.T),
            "wqk": wqk_r, "wv": wv_r, "wo": wo_r, "bo": bo,
            "maskl": zeros if j == 0 else ones,
            "maskr": zeros if j == 3 else ones,
            "ident": ident,
        })
    return in_maps


def assemble_output(results, x, Wq, Wk, Wv, Wo, bo):
    x = np.asarray(x, np.float32)
    out = np.empty((B, T, D_MODEL), np.float32)
    for core in range(8):
        b, j = divmod(core, 4)
        out[b, 1 + 1024 * j:1 + 1024 * (j + 1), :] = results[core]["outt"].T

    # global token rows, exact on host
    xg = x[:, [0, T - 1], :]                      # [B, 2, D]
    qg = (xg @ Wq.T).reshape(B, 2, H, DK) * SCALE  # [B, 2, H, DK]
    kg = (xg @ Wk.T).reshape(B, 2, H, DK)
    vg = (xg @ Wv.T).reshape(B, 2, H, DK)
    for b in range(B):
        se = np.zeros((H, 2))
        wvs = np.zeros((H, 2, DK))
        for j in range(4):
            g = results[4 * b + j]["gstats"]  # [65, 32]
            for h in range(H):
                for gi in range(2):
                    se[h, gi] += g[64, 2 * h + gi]
                    wvs[h, gi] += g[0:64, 2 * h + gi]
        # add the global-key terms: scores qg . kg
        sgg = np.einsum("ghd,fhd->hgf", qg[b], kg[b])  # [H, 2g(query), 2f(key)]
        egg = np.exp(sgg)
        num = wvs + np.einsum("hgf,fhd->hgd", egg, vg[b])
        den = se + egg.sum(-1)
        og = num / den[..., None]                  # [H, 2, DK]
        for gi, trow in ((0, 0), (1, T - 1)):
            row = og[:, gi, :].reshape(H * DK)
            out[b, trow] = row @ Wo.T + bo
    return out


def kernel(x, Wq, Wk, Wv, Wo, bo):
    nc = get_nc()
    in_maps = make_inputs(x, Wq, Wk, Wv, Wo, bo)
    res = run_bass_kernel_spmd(nc, in_maps, core_ids=list(range(8)))
    LAST["res"] = res
    results = [{k: np.asarray(v) for k, v in r.items()} for r in res.results]
    return assemble_output(results, x, Wq, Wk, Wv, Wo, bo)


# revision 28
# speedup vs baseline: 1.9175x; 1.0763x over previous
"""BigBird sparse attention kernel for 8 Trainium2 NeuronCores.

Sharding: token-parallel. B=2 batches x 4 chunks of 1024 local tokens each
-> 8 cores. Each core receives a transposed x-slice [D=1024, 1282] whose
columns are [g0, g1, 10 window blocks of 128 tokens] (blocks 8j-1 .. 8j+8,
zero-padded outside [0, 32)). The core computes:
  - q/k projections in transposed layout [f, tok] (fp32r matmuls, bf16 out)
  - v projection in [tok, f] layout with a ones-column per head
  - 3-block sliding-window attention: scores kept transposed [kt, q], exp'd
    to bf16 probabilities; the P.V matmul is FLIPPED (stationary = P block,
    moving = V||ones) so each 128-token q block lands in PSUM as
    [q, 64 v-cols + denominator] with the softmax denominator per-partition
  - attention of local tokens to the 2 global tokens (separate softmax,
    same flipped layout) -> normalize both with per-partition reciprocals,
    combine, transpose back to [feat, tok] on the PE array
  - flash-style partial stats (sum-exp, weighted V) of the 2 global query
    tokens against the core's local keys -> combined on host
  - output projection + bias for its 1024 local tokens
Host assembles the 8 slices, and computes the 2 global output rows per
batch exactly in numpy from the shipped partials.
"""

import numpy as np
import ml_dtypes

import concourse.bass as bass
import concourse.mybir as mybir
import concourse.tile as tile
from concourse import bacc
from concourse.bass_utils import run_bass_kernel_spmd

F32 = mybir.dt.float32
F32R = mybir.dt.float32r
BF16 = mybir.dt.bfloat16
AF = mybir.ActivationFunctionType
BF = ml_dtypes.bfloat16

D_MODEL = 1024
H = 16
DK = 64
BS = 128
B = 2
T = 4098
NB = 32            # global 128-blocks of local tokens
NW = 10            # window blocks per core (8 local + 2 halo)
TOKS = 2 + NW * BS # x-slice columns
SCALE = 1.0 / np.sqrt(DK)

# x-slice column layout: [g0, g1, L0..L7 (8*128 local), HL (128), HR (128)]
# q is only needed for globals + locals (cols 0:1026), k/v for everything.
CHUNKS_K = [(0, 512), (512, 512), (1024, 258)]
CHUNKS_Q = [(0, 512), (512, 512), (1024, 2)]

# pt region start per window block t (regions sized by the q-window width)
PT_START = [0, 128, 384, 768, 1152, 1536, 1920, 2304, 2688, 2944]
PT_COLS = 3072


def CK(t):
    # column of window block t in the x-slice (k/v side)
    return 1026 if t == 0 else (1154 if t == 9 else 2 + BS * (t - 1))


def CQ(qs):
    # column of local q block qs (1..8) in the x-slice
    return 2 + BS * (qs - 1)


def qlo(t):
    return max(t - 1, 1)


def ptcol(t, qs):
    # column of (window-block t, q window-position qs) in the pt tensor
    return PT_START[t] + 128 * (qs - qlo(t))


def build_kernel(nc):
    xt = nc.dram_tensor("xt", [D_MODEL, TOKS], BF16, kind="ExternalInput").ap()
    wqk = nc.dram_tensor("wqk", [2, 2, 8, 128, 512], BF16,
                         kind="ExternalInput").ap()
    wv = nc.dram_tensor("wv", [2, 8, 128, 512], BF16, kind="ExternalInput").ap()
    vgin = nc.dram_tensor("vgin", [2, 34, 768], BF16, kind="ExternalInput").ap()
    wo = nc.dram_tensor("wo", [8, 8, 128, 128], BF16, kind="ExternalInput").ap()
    bo = nc.dram_tensor("bo", [D_MODEL], F32, kind="ExternalInput").ap()
    maskl = nc.dram_tensor("maskl", [128, 1], BF16, kind="ExternalInput").ap()
    maskr = nc.dram_tensor("maskr", [128, 1], BF16, kind="ExternalInput").ap()
    ident = nc.dram_tensor("ident", [128, 128], BF16, kind="ExternalInput").ap()
    outt = nc.dram_tensor("outt", [D_MODEL, 1024], F32, kind="ExternalOutput").ap()
    gstats = nc.dram_tensor("gstats", [65, 32], F32, kind="ExternalOutput").ap()
    import os as _os
    dbg = None
    if _os.environ.get("BB_DEBUG"):
        dbg = nc.dram_tensor("dbg_at", [D_MODEL, 1024], BF16,
                             kind="ExternalOutput").ap()

    with tile.TileContext(nc) as tc:
        with (
            tc.tile_pool(name="pc", bufs=1) as pc,
            tc.tile_pool(name="px", bufs=1) as px,
            tc.tile_pool(name="pqk", bufs=1) as pqk,
            tc.tile_pool(name="pv", bufs=1) as pvp,
            tc.tile_pool(name="pwv", bufs=1) as pwv,
            tc.tile_pool(name="pw", bufs=6) as pw,
            tc.tile_pool(name="pat", bufs=1) as pat,
            tc.tile_pool(name="ppt", bufs=2) as ppt,
            tc.tile_pool(name="psm", bufs=2) as psm,
            tc.tile_pool(name="pout", bufs=2) as pout,
            tc.tile_pool(name="pssc", bufs=4, space="PSUM") as pssc,
            tc.tile_pool(name="pstl", bufs=2, space="PSUM") as pstl,
            tc.tile_pool(name="ppst", bufs=2, space="PSUM") as ppst,
        ):
            # ---- constants ----
            bo_sb = pc.tile([128, 8], F32, tag="bo")
            nc.sync.dma_start(bo_sb[:], bo.rearrange("(t p) -> p t", p=128))
            ml_sb = pc.tile([128, 1], BF16, tag="ml")
            mr_sb = pc.tile([128, 1], BF16, tag="mr")
            nc.sync.dma_start(ml_sb[:], maskl)
            nc.sync.dma_start(mr_sb[:], maskr)
            id_sb = pc.tile([128, 128], BF16, tag="ident")
            nc.sync.dma_start(id_sb[:], ident)
            gst = pc.tile([65, 32], F32, tag="gst")

            def wqk_dma_pair(pss, half, dp, name):
                # [128, 1024] tile covering d=2dp (cols 0:512) and d=2dp+1;
                # each 512-col block is [q ft0 | q ft1 | k ft0 | k ft1]
                wt = pw.tile([128, 1024], BF16, tag="w", bufs=8, name=name)
                src = bass.AP(wqk.tensor, wqk[pss, half, 2 * dp].offset,
                              [[512, 128], [128 * 512, 2], [1, 512]])
                nc.sync.dma_start(wt[:], src)
                return wt

            def wqk_pre(pss, half):
                return [wqk_dma_pair(pss, half, dp, f"wt{pss}{half}_{dp}")
                        for dp in range(4)]

            # ---- x slice, transposed, resident; first-pass q/k weights
            # interleave so the projection d-loop starts immediately ----
            xts, wts0 = [], []
            for dp in range(4):
                wts0.append(wqk_dma_pair(0, 0, dp, f"wt00_{dp}"))
                for d in (2 * dp, 2 * dp + 1):
                    xd = px.tile([128, TOKS], BF16, tag=f"xt{d}")
                    nc.sync.dma_start(xd[:], xt[128 * d:128 * (d + 1), :])
                    xts.append(xd)

            at_sb = [pat.tile([128, 1024], BF16, tag=f"at{f}", name=f"at{f}")
                     for f in range(8)]

            def proj_psum(i2, c, cn):
                # 6 simultaneous 1-bank accumulators: 4 from pssc, 2 from pstl
                pool = pssc if 3 * i2 + c < 4 else pstl
                return pool.tile([128, cn], F32, tag="ps1" if pool is pssc
                                 else "pstl", name=f"pj{i2}_{c}")

            def emit_qk_proj(pss, half, qk_tiles, pre=None):
                # SCALE is folded into the q weights host-side
                wts = pre if pre is not None else wqk_pre(pss, half)
                for pi, pname in enumerate(("q", "k")):
                    chunks = CHUNKS_Q if pname == "q" else CHUNKS_K
                    osbs, psjs = [], []
                    for i2 in range(2):
                        i = 2 * half + i2
                        osb = pqk.tile([128, TOKS], BF16, tag=f"qk{pname}{i}",
                                       name=f"qk{pname}{i}")
                        qk_tiles[(pname, i)] = osb
                        osbs.append(osb)
                        psjs.append([proj_psum(i2, c, cn)
                                     for c, (c0, cn) in enumerate(chunks)])
                    for d in range(8):
                        wcol = 512 * (d % 2) + 256 * pi
                        for i2 in range(2):
                            for c, (c0, cn) in enumerate(chunks):
                                nc.tensor.matmul(
                                    psjs[i2][c][:, :cn],
                                    wts[d // 2][:, wcol + 128 * i2:
                                                wcol + 128 * i2 + 128],
                                    xts[d][:, c0:c0 + cn],
                                    start=(d == 0), stop=(d == 7))
                    # evictions split across VEC (q) and ACT (k) so the next
                    # d-loop's PSUM slots free without queue backlog
                    for i2 in range(2):
                        for c, (c0, cn) in enumerate(chunks):
                            if pname == "q":
                                nc.vector.tensor_copy(osbs[i2][:, c0:c0 + cn],
                                                      psjs[i2][c][:, :cn])
                            else:
                                nc.scalar.copy(osbs[i2][:, c0:c0 + cn],
                                               psjs[i2][c][:, :cn])

            def emit_v_proj(pss):
                wv_sb = []
                for d in range(8):
                    wvd = pwv.tile([128, 512], BF16, tag=f"wv{d}", name=f"wv{d}")
                    nc.sync.dma_start(wvd[:], wv[pss, d])
                    wv_sb.append(wvd)
                v96 = []
                for tb in range(NW):
                    pv_ps = pssc.tile([128, 512], F32, tag="ps1", name="pv_ps")
                    for d in range(8):
                        nc.tensor.matmul(pv_ps[:], xts[d][:, CK(tb):CK(tb) + 128],
                                         wv_sb[d][:], start=(d == 0), stop=(d == 7))
                    vt = pvp.tile([128, 8 * 96], BF16, tag=f"v96_{tb}",
                                  name=f"v96_{tb}")
                    pstep = vt.ap[0][0]
                    dst = bass.AP(vt.tensor, vt[:].offset,
                                  [[pstep, 128], [96, 8], [1, 64]])
                    src = bass.AP(pv_ps.tensor, pv_ps[:].offset,
                                  [[pv_ps.ap[0][0], 128], [64, 8], [1, 64]])
                    nc.vector.tensor_copy(dst, src)
                    onesap = bass.AP(vt.tensor, vt[:].offset + 64,
                                     [[pstep, 128], [96, 8], [1, 1]])
                    nc.vector.memset(onesap, 1.0)
                    v96.append(vt)
                # vg (global-token V rows + ones col) comes exact from the
                # host, duplicated at partitions 0:2 and 32:34 to match the
                # paired-pxg stationary placement
                vg_sb = pvp.tile([34, 8 * 96], BF16, tag="vg", name="vg")
                nc.sync.dma_start(vg_sb[:], vgin[pss])
                return v96, vg_sb

            pxg_pairs = {}

            def emit_scores(h, qk_tiles):
                """Scores + exps for head h: pt [kt, q], paired pxg, pg."""
                hl = h % 8
                r0 = 64 * (hl % 2)
                qt = qk_tiles[("q", hl // 2)]
                kt_ = qk_tiles[("k", hl // 2)]
                qh = qt[r0:r0 + 64, :]
                kh = kt_[r0:r0 + 64, :]

                if h % 2 == 0:
                    # local-q -> global-k scores for BOTH heads of the pair in
                    # one matmul per 512-token chunk: stationary [128, 34] with
                    # head-even kg in (rows 0:64, cols 0:2) and head-odd kg in
                    # (rows 64:128, cols 32:34); zeros elsewhere mask the
                    # cross-head terms.
                    kg2 = psm.tile([128, 34], BF16, tag="kg2", name="kg2")
                    nc.vector.memset(kg2[:], 0.0)
                    nc.vector.tensor_copy(kg2[0:64, 0:2], kt_[0:64, 0:2])
                    nc.vector.tensor_copy(kg2[64:128, 32:34], kt_[64:128, 0:2])
                    pxg2 = psm.tile([34, 1024], BF16, tag="pxg", name="pxg2",
                                    bufs=2)
                    for c in range(2):
                        ps_xg = pssc.tile([34, 512], F32, tag="ps1",
                                          name="ps_xg")
                        nc.tensor.matmul(ps_xg[:], kg2[:],
                                         qt[:, 2 + 512 * c:2 + 512 * c + 512],
                                         start=True, stop=True)
                        nc.scalar.activation(pxg2[:, 512 * c:512 * c + 512],
                                             ps_xg[:], AF.Exp)
                    pxg_pairs[h // 2] = pxg2
                pxg2 = pxg_pairs[h // 2]

                # window scores [kt, q] with the global-q scores (psg) riding
                # each k-block's stationary; edge blocks share a psum bank so
                # the exp runs as one ACT instruction
                psg = pssc.tile([128, 16], F32, tag="ps1", name="psg")
                pt = ppt.tile([128, PT_COLS], BF16, tag="pt", name="pt")
                ps_e = pssc.tile([128, 384], F32, tag="ps1", name="ps_e")
                nc.tensor.matmul(ps_e[:, 0:128], kh[:, CK(0):CK(0) + 128],
                                 qh[:, CQ(1):CQ(1) + 128], start=True, stop=True)
                nc.tensor.matmul(ps_e[:, 128:384], kh[:, CK(1):CK(1) + 128],
                                 qh[:, CQ(1):CQ(1) + 256], start=True, stop=True)
                nc.tensor.matmul(psg[:, 0:2], kh[:, CK(1):CK(1) + 128],
                                 qh[:, 0:2], start=True, stop=False)
                nc.scalar.activation(pt[:, 0:384], ps_e[:], AF.Exp)
                for t in range(2, 8):
                    ps_s = pssc.tile([128, 384], F32, tag="ps1", name="ps_s")
                    nc.tensor.matmul(ps_s[:], kh[:, CK(t):CK(t) + 128],
                                     qh[:, CQ(t - 1):CQ(t - 1) + 384],
                                     start=True, stop=True)
                    nc.tensor.matmul(psg[:, 2 * (t - 1):2 * t],
                                     kh[:, CK(t):CK(t) + 128], qh[:, 0:2],
                                     start=False, stop=False)
                    nc.scalar.activation(pt[:, PT_START[t]:PT_START[t] + 384],
                                         ps_s[:], AF.Exp)
                ps_f = pssc.tile([128, 384], F32, tag="ps1", name="ps_f")
                nc.tensor.matmul(ps_f[:, 0:256], kh[:, CK(8):CK(8) + 128],
                                 qh[:, CQ(7):CQ(7) + 256], start=True, stop=True)
                nc.tensor.matmul(psg[:, 14:16], kh[:, CK(8):CK(8) + 128],
                                 qh[:, 0:2], start=False, stop=True)
                nc.tensor.matmul(ps_f[:, 256:384], kh[:, CK(9):CK(9) + 128],
                                 qh[:, CQ(8):CQ(8) + 128], start=True, stop=True)
                pg = psm.tile([128, 16], BF16, tag="pgsb", name="pg", bufs=3)
                nc.scalar.activation(pg[:], psg[:], AF.Exp)
                nc.scalar.activation(pt[:, 2688:3072], ps_f[:], AF.Exp)
                # edge masks: first/last local block of the batch row
                nc.gpsimd.tensor_mul(pt[:, 0:128], pt[:, 0:128],
                                     ml_sb[:].to_broadcast((128, 128)))
                nc.gpsimd.tensor_mul(pt[:, 2944:3072], pt[:, 2944:3072],
                                     mr_sb[:].to_broadcast((128, 128)))
                return {"pt": pt, "pxg2": pxg2, "pg": pg}

            def emit_tail(h, S, v96, vg_sb, psT_pair):
                """Flipped P.V + normalize + transpose for head h."""
                hl = h % 8
                r0 = 64 * (hl % 2)
                r2 = 32 * (h % 2)
                pt, pxg2, pg = S["pt"], S["pxg2"], S["pg"]
                for c in range(2):
                    psA = pstl.tile([128, 260], F32, tag="pstl", name="psA")
                    psB = pstl.tile([128, 260], F32, tag="pstl", name="psB")
                    for j in range(4):
                        qs = 4 * c + j + 1
                        win = (qs - 1, qs, qs + 1)
                        for i, t in enumerate(win):
                            nc.tensor.matmul(
                                psA[:, 65 * j:65 * j + 65],
                                pt[:, ptcol(t, qs):ptcol(t, qs) + 128],
                                v96[t][:, 96 * hl:96 * hl + 65],
                                start=(i == 0), stop=(i == 2))
                        nc.tensor.matmul(
                            psB[:, 65 * j:65 * j + 65],
                            pxg2[r2:r2 + 2, 128 * (qs - 1):128 * qs],
                            vg_sb[r2:r2 + 2, 96 * hl:96 * hl + 65],
                            start=True, stop=True)
                    pA = psA.ap[0][0]
                    pB = psB.ap[0][0]
                    rA = psm.tile([128, 4], F32, tag="rA", name="rA")
                    rB = psm.tile([128, 4], F32, tag="rB", name="rB")
                    nc.vector.reciprocal(
                        rA[:], bass.AP(psA.tensor, psA[:].offset + 64,
                                       [[pA, 128], [65, 4]]))
                    nc.vector.reciprocal(
                        rB[:], bass.AP(psB.tensor, psB[:].offset + 64,
                                       [[pB, 128], [65, 4]]))
                    numA = bass.AP(psA.tensor, psA[:].offset,
                                   [[pA, 128], [65, 4], [1, 64]])
                    numB = bass.AP(psB.tensor, psB[:].offset,
                                   [[pB, 128], [65, 4], [1, 64]])
                    rAb = bass.AP(rA.tensor, rA[:].offset,
                                  [[rA.ap[0][0], 128], [1, 4], [0, 64]])
                    rBb = bass.AP(rB.tensor, rB[:].offset,
                                  [[rB.ap[0][0], 128], [1, 4], [0, 64]])
                    tmpA = psm.tile([128, 256], F32, tag="tmpA", name="tmpA")
                    tmpB = psm.tile([128, 256], F32, tag="tmpB", name="tmpB")
                    tA = bass.AP(tmpA.tensor, tmpA[:].offset,
                                 [[tmpA.ap[0][0], 128], [64, 4], [1, 64]])
                    tB = bass.AP(tmpB.tensor, tmpB[:].offset,
                                 [[tmpB.ap[0][0], 128], [64, 4], [1, 64]])
                    nc.vector.tensor_mul(tA, numA, rAb)
                    nc.vector.tensor_mul(tB, numB, rBb)
                    atq = psm.tile([128, 256], BF16, tag="atq", name="atq")
                    nc.gpsimd.tensor_add(atq[:], tmpA[:], tmpB[:])
                    for j in range(4):
                        nc.tensor.transpose(
                            psT_pair[c][r0:r0 + 64, 128 * j:128 * j + 128],
                            atq[:, 64 * j:64 * j + 64], id_sb[:])
                # flash partials of the 2 global queries vs this core's keys
                ps_wv = pstl.tile([65, 2], F32, tag="pstl", name="ps_wv")
                for t in range(1, 9):
                    nc.tensor.matmul(ps_wv[:], v96[t][:, 96 * hl:96 * hl + 65],
                                     pg[:, 2 * (t - 1):2 * t],
                                     start=(t == 1), stop=(t == 8))
                nc.vector.tensor_copy(gst[:, 2 * h:2 * h + 2], ps_wv[:])
                if hl % 2 == 1:
                    for c in range(2):
                        nc.vector.tensor_copy(
                            at_sb[h // 2][:, 512 * c:512 * c + 512],
                            psT_pair[c][:])

            # software-pipelined emission: scores of head h+1 are emitted
            # before the tail of head h; pass-B projections interleave at
            # group boundaries (PE executes in program order)
            qk0, qk1 = {}, {}
            emit_qk_proj(0, 0, qk0, pre=wts0)
            emit_qk_proj(0, 1, qk0)
            v96_0, vg0 = emit_v_proj(0)
            v96_1, vg1 = None, None
            S = {0: emit_scores(0, qk0)}
            psT_pair = None
            pre10 = pre11 = None
            for h in range(16):
                if h + 1 < 16:
                    S[h + 1] = emit_scores(h + 1, qk0 if h + 1 < 8 else qk1)
                if h % 2 == 0:
                    psT_pair = [ppst.tile([128, 512], BF16, tag="psT",
                                          name=f"psT{h}_{c}")
                                for c in range(2)]
                v96, vg = (v96_0, vg0) if h < 8 else (v96_1, vg1)
                emit_tail(h, S.pop(h), v96, vg, psT_pair)
                if h == 0:
                    pre10 = wqk_pre(1, 0)
                if h == 2:
                    emit_qk_proj(1, 0, qk1, pre=pre10)
                if h == 4:
                    pre11 = wqk_pre(1, 1)
                if h == 6:
                    emit_qk_proj(1, 1, qk1, pre=pre11)
                if h == 7:
                    v96_1, vg1 = emit_v_proj(1)

            # ================= output projection =================
            # flash partials for the host-side global rows can ship now
            nc.sync.dma_start(gstats, gst[:])
            # prefetch ALL weight tiles before the barrier so their DMAs
            # land during the attention tail
            wot_pre = []
            for m in range(8):
                wotp = pw.tile([128, 1024], BF16, tag="wo", bufs=8,
                               name=f"wot{m}")
                wsrc = bass.AP(wo.tensor, wo[m, 0].offset,
                               [[128, 128], [128 * 128, 8], [1, 128]])
                nc.sync.dma_start(wotp[:], wsrc)
                wot_pre.append(wotp)
            tc.no_sync_barrier()
            for m in range(8):
                # alternate PSUM pools and eviction engines across m so the
                # accumulate->bias->DMA chain of consecutive tiles pipelines
                pool = pssc if m % 2 == 0 else pstl
                ps_op = [pool.tile([128, 512], F32,
                                   tag="ps1" if m % 2 == 0 else "pstl",
                                   name=f"pop{c}")
                         for c in range(2)]
                wot = wot_pre[m]
                for f in range(8):
                    for c in range(2):
                        nc.tensor.matmul(ps_op[c][:], wot[:, 128 * f:128 * f + 128],
                                         at_sb[f][:, 512 * c:512 * c + 512],
                                         start=(f == 0), stop=(f == 7))
                for c in range(2):
                    ot = pout.tile([128, 512], F32, tag="ot", bufs=4)
                    if c == 0:
                        nc.scalar.activation(ot[:], ps_op[c][:], AF.Identity,
                                             bias=bo_sb[:, m:m + 1])
                    else:
                        nc.vector.tensor_scalar_add(ot[:], ps_op[c][:],
                                                    bo_sb[:, m:m + 1])
                    nc.sync.dma_start(outt[128 * m:128 * (m + 1),
                                           512 * c:512 * c + 512], ot[:])
            if dbg is not None:
                for f in range(8):
                    nc.sync.dma_start(dbg[128 * f:128 * (f + 1), :], at_sb[f][:])
    return nc


_NC_CACHE = {}
LAST = {}


def get_nc():
    if "nc" not in _NC_CACHE:
        nc = bacc.Bacc("TRN2", target_bir_lowering=False, debug=False, num_devices=8)
        build_kernel(nc)
        nc.compile()
        _NC_CACHE["nc"] = nc
    return _NC_CACHE["nc"]


def make_inputs(x, Wq, Wk, Wv, Wo, bo):
    """Build the 8 per-core input maps (all host-side numpy)."""
    x = np.asarray(x, np.float32)
    Wq = np.asarray(Wq, np.float32)
    Wk = np.asarray(Wk, np.float32)
    Wv = np.asarray(Wv, np.float32)
    Wo = np.asarray(Wo, np.float32)
    bo = np.asarray(bo, np.float32)

    wq_r = (Wq * SCALE).T.reshape(8, 128, 8, 128).transpose(2, 0, 1, 3)
    wk_r = Wk.T.reshape(8, 128, 8, 128).transpose(2, 0, 1, 3)  # [ft, d, 128d, 128f]
    # [pss, half, d, 128d, 512]: per (pass, half, d) the 512 cols are
    # [q ft0 | q ft1 | k ft0 | k ft1]; SCALE folded into q
    qp = wq_r.reshape(2, 2, 2, 8, 128, 128).transpose(0, 1, 3, 4, 2, 5)
    kp = wk_r.reshape(2, 2, 2, 8, 128, 128).transpose(0, 1, 3, 4, 2, 5)
    wqk_r = np.ascontiguousarray(np.concatenate(
        [qp.reshape(2, 2, 8, 128, 256), kp.reshape(2, 2, 8, 128, 256)],
        -1)).astype(BF)
    wv_r = np.ascontiguousarray(
        Wv.T.reshape(8, 128, 2, 512).transpose(2, 0, 1, 3)).astype(BF)
    wo_r = np.ascontiguousarray(
        Wo.T.reshape(8, 128, 8, 128).transpose(2, 0, 1, 3)).astype(BF)
    # wo_r[m, f, i, j] must be Wo[128m+j, 128f+i] = Wo.T[128f+i, 128m+j]

    ones = np.ones((128, 1), BF)
    zeros = np.zeros((128, 1), BF)
    ident = np.eye(128, dtype=BF)
    in_maps = []
    for core in range(8):
        b, j = divmod(core, 4)
        # x-slice columns: [g0, g1, L0..L7, halo-left, halo-right]
        xs = np.zeros((TOKS, D_MODEL), np.float32)
        xs[0] = x[b, 0]
        xs[1] = x[b, T - 1]
        for w in range(NW):
            gb = 8 * j - 1 + w
            col = 1026 if w == 0 else (1154 if w == 9 else 2 + 128 * (w - 1))
            if 0 <= gb < NB:
                xs[col:col + 128] = x[b, 1 + 128 * gb:1 + 128 * (gb + 1)]
        # exact global-token V rows (+ ones col) in v96 layout, duplicated at
        # partitions 0:2 and 32:34
        vgb = xs[0:2] @ Wv.T  # [2, 1024]
        vgi = np.zeros((2, 34, 768), np.float32)
        for p in range(2):
            for hl in range(8):
                vgi[p, 0:2, 96 * hl:96 * hl + 64] = \
                    vgb[:, 512 * p + 64 * hl:512 * p + 64 * hl + 64]
                vgi[p, 0:2, 96 * hl + 64] = 1.0
            vgi[p, 32:34] = vgi[p, 0:2]
        in_maps.append({
            "xt": np.ascontiguousarray(xs.T).astype(BF),
            "wqk": wqk_r, "wv": wv_r, "wo": wo_r, "bo": bo,
            "maskl": zeros if j == 0 else ones,
            "maskr": zeros if j == 3 else ones,
            "ident": ident,
            "vgin": vgi.astype(BF),
        })
    return in_maps


def assemble_output(results, x, Wq, Wk, Wv, Wo, bo):
    x = np.asarray(x, np.float32)
    out = np.empty((B, T, D_MODEL), np.float32)
    for core in range(8):
        b, j = divmod(core, 4)
        out[b, 1 + 1024 * j:1 + 1024 * (j + 1), :] = results[core]["outt"].T

    # global token rows, exact on host
    xg = x[:, [0, T - 1], :]                      # [B, 2, D]
    qg = (xg @ Wq.T).reshape(B, 2, H, DK) * SCALE  # [B, 2, H, DK]
    kg = (xg @ Wk.T).reshape(B, 2, H, DK)
    vg = (xg @ Wv.T).reshape(B, 2, H, DK)
    for b in range(B):
        se = np.zeros((H, 2))
        wvs = np.zeros((H, 2, DK))
        for j in range(4):
            g = results[4 * b + j]["gstats"]  # [65, 32]
            for h in range(H):
                for gi in range(2):
                    se[h, gi] += g[64, 2 * h + gi]
                    wvs[h, gi] += g[0:64, 2 * h + gi]
        # add the global-key terms: scores qg . kg
        sgg = np.einsum("ghd,fhd->hgf", qg[b], kg[b])  # [H, 2g(query), 2f(key)]
        egg = np.exp(sgg)
        num = wvs + np.einsum("hgf,fhd->hgd", egg, vg[b])
        den = se + egg.sum(-1)
        og = num / den[..., None]                  # [H, 2, DK]
        for gi, trow in ((0, 0), (1, T - 1)):
            row = og[:, gi, :].reshape(H * DK)
            out[b, trow] = row @ Wo.T + bo
    return out


def kernel(x, Wq, Wk, Wv, Wo, bo):
    nc = get_nc()
    in_maps = make_inputs(x, Wq, Wk, Wv, Wo, bo)
    res = run_bass_kernel_spmd(nc, in_maps, core_ids=list(range(8)))
    LAST["res"] = res
    results = [{k: np.asarray(v) for k, v in r.items()} for r in res.results]
    return assemble_output(results, x, Wq, Wk, Wv, Wo, bo)


# revision 30
# speedup vs baseline: 2.0388x; 1.0632x over previous
"""BigBird sparse attention kernel for 8 Trainium2 NeuronCores.

Sharding: token-parallel. B=2 batches x 4 chunks of 1024 local tokens each
-> 8 cores. Each core receives a transposed bf16 x-slice [D=1024, 1280]
whose columns are [8 local 128-token blocks, halo-left, halo-right]
(halos zero-padded outside [0, 32)). Global-token q/k/v rows are computed
exactly on the host and shipped as tiny side inputs. The core computes:
  - q/k projections in transposed layout [f, tok] (bf16 matmuls, SCALE
    folded into the q weights host-side)
  - v projection in [tok, f] layout with a ones-column per head
  - 3-block sliding-window attention: scores kept transposed [kt, q],
    exp'd to bf16 probabilities (middle k-blocks share 2-bank psum tiles
    so two blocks exp in one ACT instruction); the P.V matmul is FLIPPED
    (stationary = P block, moving = V||ones) so each 128-token q block
    lands in PSUM as [q, 64 v-cols + denominator] with the softmax
    denominator per-partition
  - attention of local tokens to the 2 global tokens (separate softmax,
    paired across heads via a zero-masked [128, 34] stationary) ->
    normalize both with per-partition reciprocals, combine, transpose
    back to [feat, tok] on the PE array
  - flash-style partial stats (sum-exp, weighted V) of the 2 global query
    tokens against the core's local keys -> combined on host
  - output projection + bias for its 1024 local tokens
Host assembles the 8 slices, and computes the 2 global output rows per
batch exactly in numpy from the shipped partials.
"""

import numpy as np
import ml_dtypes

import concourse.bass as bass
import concourse.mybir as mybir
import concourse.tile as tile
from concourse import bacc
from concourse.bass_utils import run_bass_kernel_spmd

F32 = mybir.dt.float32
BF16 = mybir.dt.bfloat16
AF = mybir.ActivationFunctionType
BF = ml_dtypes.bfloat16

D_MODEL = 1024
H = 16
DK = 64
BS = 128
B = 2
T = 4098
NB = 32            # global 128-blocks of local tokens
NW = 10            # window blocks per core (8 local + 2 halo)
TOKS = NW * BS     # x-slice columns: [L0..L7, halo-left, halo-right]
SCALE = 1.0 / np.sqrt(DK)

CHUNKS_Q = [(0, 512), (512, 512)]
CHUNKS_K = [(0, 512), (512, 512), (1024, 256)]

# pt region start per window block t; middle blocks are exp'd in pairs from
# 2-bank psum tiles, leaving 128-col junk gaps (cols 768.., 1664.., 2560..)
PT_START = [0, 128, 384, 896, 1280, 1792, 2176, 2688, 3072, 3328]
PT_COLS = 3456


def CK(t):
    # column of window block t in the x-slice (k/v side)
    return 1024 if t == 0 else (1152 if t == 9 else BS * (t - 1))


def CQ(qs):
    # column of local q block qs (1..8) in the x-slice
    return BS * (qs - 1)


def qlo(t):
    return max(t - 1, 1)


def ptcol(t, qs):
    # column of (window-block t, q window-position qs) in the pt tensor
    return PT_START[t] + 128 * (qs - qlo(t))


def build_kernel(nc):
    xt = nc.dram_tensor("xt", [D_MODEL, TOKS], BF16, kind="ExternalInput").ap()
    wqk = nc.dram_tensor("wqk", [2, 2, 8, 128, 512], BF16,
                         kind="ExternalInput").ap()
    wv = nc.dram_tensor("wv", [2, 8, 128, 512], BF16, kind="ExternalInput").ap()
    vgin = nc.dram_tensor("vgin", [2, 34, 768], BF16, kind="ExternalInput").ap()
    qgin = nc.dram_tensor("qgin", [2, 128, 8], BF16, kind="ExternalInput").ap()
    kgin = nc.dram_tensor("kgin", [2, 128, 8], BF16, kind="ExternalInput").ap()
    wo = nc.dram_tensor("wo", [8, 8, 128, 128], BF16, kind="ExternalInput").ap()
    bo = nc.dram_tensor("bo", [D_MODEL], F32, kind="ExternalInput").ap()
    maskl = nc.dram_tensor("maskl", [128, 1], BF16, kind="ExternalInput").ap()
    maskr = nc.dram_tensor("maskr", [128, 1], BF16, kind="ExternalInput").ap()
    ident = nc.dram_tensor("ident", [128, 128], BF16, kind="ExternalInput").ap()
    outt = nc.dram_tensor("outt", [D_MODEL, 1024], F32, kind="ExternalOutput").ap()
    gstats = nc.dram_tensor("gstats", [65, 32], F32, kind="ExternalOutput").ap()
    import os as _os
    dbg = None
    if _os.environ.get("BB_DEBUG"):
        dbg = nc.dram_tensor("dbg_at", [D_MODEL, 1024], BF16,
                             kind="ExternalOutput").ap()

    with tile.TileContext(nc) as tc:
        with (
            tc.tile_pool(name="pc", bufs=1) as pc,
            tc.tile_pool(name="px", bufs=1) as px,
            tc.tile_pool(name="pqk", bufs=1) as pqk,
            tc.tile_pool(name="pv", bufs=1) as pvp,
            tc.tile_pool(name="pwv", bufs=1) as pwv,
            tc.tile_pool(name="pw", bufs=6) as pw,
            tc.tile_pool(name="pat", bufs=1) as pat,
            tc.tile_pool(name="ppt", bufs=2) as ppt,
            tc.tile_pool(name="psm", bufs=2) as psm,
            tc.tile_pool(name="pout", bufs=2) as pout,
            # PSUM: 2x 2-bank pair slots + 1 psg bank + 2x 1-bank + 1 psT
            tc.tile_pool(name="pspr", bufs=2, space="PSUM") as pspr,
            tc.tile_pool(name="ppsg", bufs=1, space="PSUM") as ppsg,
            tc.tile_pool(name="pstl", bufs=2, space="PSUM") as pstl,
            tc.tile_pool(name="ppst", bufs=1, space="PSUM") as ppst,
        ):
            # ---- constants ----
            bo_sb = pc.tile([128, 8], F32, tag="bo")
            nc.sync.dma_start(bo_sb[:], bo.rearrange("(t p) -> p t", p=128))
            ml_sb = pc.tile([128, 1], BF16, tag="ml")
            mr_sb = pc.tile([128, 1], BF16, tag="mr")
            nc.sync.dma_start(ml_sb[:], maskl)
            nc.sync.dma_start(mr_sb[:], maskr)
            id_sb = pc.tile([128, 128], BF16, tag="ident")
            nc.sync.dma_start(id_sb[:], ident)
            gst = pc.tile([65, 32], F32, tag="gst")

            def wqk_dma_pair(pss, half, dp, name):
                # [128, 1024] tile covering d=2dp (cols 0:512) and d=2dp+1;
                # each 512-col block is [q ft0 | q ft1 | k ft0 | k ft1]
                wt = pw.tile([128, 1024], BF16, tag="w", bufs=8, name=name)
                src = bass.AP(wqk.tensor, wqk[pss, half, 2 * dp].offset,
                              [[512, 128], [128 * 512, 2], [1, 512]])
                nc.sync.dma_start(wt[:], src)
                return wt

            def wqk_pre(pss, half):
                return [wqk_dma_pair(pss, half, dp, f"wt{pss}{half}_{dp}")
                        for dp in range(4)]

            # ---- x slice, transposed, resident; first-pass q/k weights
            # interleave so the projection d-loop starts immediately ----
            xts, wts0 = [], []
            for dp in range(4):
                wts0.append(wqk_dma_pair(0, 0, dp, f"wt00_{dp}"))
                for d in (2 * dp, 2 * dp + 1):
                    xd = px.tile([128, TOKS], BF16, tag=f"xt{d}")
                    nc.sync.dma_start(xd[:], xt[128 * d:128 * (d + 1), :])
                    xts.append(xd)

            at_sb = [pat.tile([128, 1024], BF16, tag=f"at{f}", name=f"at{f}")
                     for f in range(8)]

            GQK = {}

            def emit_qk_proj(pss, half, qk_tiles, pre=None):
                # SCALE is folded into the q weights host-side
                wts = pre if pre is not None else wqk_pre(pss, half)
                if half == 0:
                    qg_sb = psm.tile([128, 8], BF16, tag="qg", name="qg_sb")
                    kg_sb = psm.tile([128, 8], BF16, tag="kg", name="kg_sb")
                    nc.sync.dma_start(qg_sb[:], qgin[pss])
                    nc.sync.dma_start(kg_sb[:], kgin[pss])
                    GQK[pss] = (qg_sb, kg_sb)
                for pi, pname in enumerate(("q", "k")):
                    chunks = CHUNKS_Q if pname == "q" else CHUNKS_K
                    osbs, psjs = [], []
                    for i2 in range(2):
                        i = 2 * half + i2
                        osb = pqk.tile([128, TOKS], BF16, tag=f"qk{pname}{i}",
                                       name=f"qk{pname}{i}")
                        qk_tiles[(pname, i)] = osb
                        osbs.append(osb)
                        # chunks 0+1 pack into one 2-bank pair tile; the k
                        # halo chunk gets a 1-bank tile
                        pj = [pspr.tile([128, 1024], F32, tag="pspr",
                                        name=f"pj{i2}")]
                        if pname == "k":
                            pj.append(pstl.tile([128, 256], F32, tag="pstl",
                                                name=f"pjh{i2}"))
                        psjs.append(pj)
                    for d in range(8):
                        wcol = 512 * (d % 2) + 256 * pi
                        for i2 in range(2):
                            for c, (c0, cn) in enumerate(chunks):
                                dst = (psjs[i2][0][:, c0:c0 + cn] if c < 2
                                       else psjs[i2][1][:, 0:cn])
                                nc.tensor.matmul(
                                    dst,
                                    wts[d // 2][:, wcol + 128 * i2:
                                                wcol + 128 * i2 + 128],
                                    xts[d][:, c0:c0 + cn],
                                    start=(d == 0), stop=(d == 7))
                    # evictions split across VEC (q) and ACT (k) so the next
                    # d-loop's PSUM slots free without queue backlog
                    for i2 in range(2):
                        if pname == "q":
                            nc.vector.tensor_copy(osbs[i2][:, 0:1024],
                                                  psjs[i2][0][:])
                        else:
                            nc.scalar.copy(osbs[i2][:, 0:1024], psjs[i2][0][:])
                            nc.scalar.copy(osbs[i2][:, 1024:1280],
                                           psjs[i2][1][:])
                return qk_tiles

            def emit_v_proj(pss):
                wv_sb = []
                for d in range(8):
                    wvd = pwv.tile([128, 512], BF16, tag=f"wv{d}", name=f"wv{d}")
                    nc.sync.dma_start(wvd[:], wv[pss, d])
                    wv_sb.append(wvd)
                v96 = []
                for tb in range(NW):
                    pv_ps = pstl.tile([128, 512], F32, tag="pstl", name="pv_ps")
                    for d in range(8):
                        nc.tensor.matmul(pv_ps[:], xts[d][:, CK(tb):CK(tb) + 128],
                                         wv_sb[d][:], start=(d == 0), stop=(d == 7))
                    vt = pvp.tile([128, 8 * 96], BF16, tag=f"v96_{tb}",
                                  name=f"v96_{tb}")
                    pstep = vt.ap[0][0]
                    dst = bass.AP(vt.tensor, vt[:].offset,
                                  [[pstep, 128], [96, 8], [1, 64]])
                    src = bass.AP(pv_ps.tensor, pv_ps[:].offset,
                                  [[pv_ps.ap[0][0], 128], [64, 8], [1, 64]])
                    nc.vector.tensor_copy(dst, src)
                    onesap = bass.AP(vt.tensor, vt[:].offset + 64,
                                     [[pstep, 128], [96, 8], [1, 1]])
                    nc.vector.memset(onesap, 1.0)
                    v96.append(vt)
                # vg (global-token V rows + ones col) comes exact from the
                # host, duplicated at partitions 0:2 and 32:34 to match the
                # paired-pxg stationary placement
                vg_sb = pvp.tile([34, 8 * 96], BF16, tag="vg", name="vg")
                nc.sync.dma_start(vg_sb[:], vgin[pss])
                return v96, vg_sb

            pxg_pairs = {}

            def emit_scores(h, qk_tiles):
                """Scores + exps for head h: pt [kt, q], paired pxg, pg."""
                hl = h % 8
                r0 = 64 * (hl % 2)
                i = hl // 2
                qt = qk_tiles[("q", i)]
                kt_ = qk_tiles[("k", i)]
                qh = qt[r0:r0 + 64, :]
                kh = kt_[r0:r0 + 64, :]
                qg_sb, kg_sb = GQK[h // 8]
                qg = qg_sb[r0:r0 + 64, 2 * i:2 * i + 2]

                if h % 2 == 0:
                    # local-q -> global-k scores for BOTH heads of the pair in
                    # one matmul per 512-token chunk: stationary [128, 34] with
                    # head-even kg in (rows 0:64, cols 0:2) and head-odd kg in
                    # (rows 64:128, cols 32:34); zeros elsewhere mask the
                    # cross-head terms.
                    kg2 = psm.tile([128, 34], BF16, tag="kg2", name="kg2")
                    nc.vector.memset(kg2[:], 0.0)
                    nc.vector.tensor_copy(kg2[0:64, 0:2],
                                          kg_sb[0:64, 2 * i:2 * i + 2])
                    nc.vector.tensor_copy(kg2[64:128, 32:34],
                                          kg_sb[64:128, 2 * i:2 * i + 2])
                    pxg2 = psm.tile([34, 1024], BF16, tag="pxg", name="pxg2",
                                    bufs=2)
                    for c in range(2):
                        ps_xg = pstl.tile([34, 512], F32, tag="pstl",
                                          name="ps_xg")
                        nc.tensor.matmul(ps_xg[:], kg2[:],
                                         qt[:, 512 * c:512 * c + 512],
                                         start=True, stop=True)
                        nc.scalar.activation(pxg2[:, 512 * c:512 * c + 512],
                                             ps_xg[:], AF.Exp)
                    pxg_pairs[h // 2] = pxg2
                pxg2 = pxg_pairs[h // 2]

                # window scores [kt, q]; edge blocks first (their 1-bank psum
                # tiles free early for the following tail), middle blocks in
                # 2-bank pairs exp'd by a single ACT instruction each. The
                # global-q scores (psg, own bank) ride each k-block stationary.
                psg = ppsg.tile([128, 16], F32, tag="psg", name="psg")
                pt = ppt.tile([128, PT_COLS], BF16, tag="pt", name="pt")
                e01 = pstl.tile([128, 384], F32, tag="pstl", name="e01")
                nc.tensor.matmul(e01[:, 0:128], kh[:, CK(0):CK(0) + 128],
                                 qh[:, CQ(1):CQ(1) + 128], start=True, stop=True)
                nc.tensor.matmul(e01[:, 128:384], kh[:, CK(1):CK(1) + 128],
                                 qh[:, CQ(1):CQ(1) + 256], start=True, stop=True)
                nc.tensor.matmul(psg[:, 0:2], kh[:, CK(1):CK(1) + 128],
                                 qg, start=True, stop=False)
                nc.scalar.activation(pt[:, 0:384], e01[:], AF.Exp)
                e89 = pstl.tile([128, 384], F32, tag="pstl", name="e89")
                nc.tensor.matmul(e89[:, 0:256], kh[:, CK(8):CK(8) + 128],
                                 qh[:, CQ(7):CQ(7) + 256], start=True, stop=True)
                nc.tensor.matmul(psg[:, 14:16], kh[:, CK(8):CK(8) + 128],
                                 qg, start=False, stop=False)
                nc.tensor.matmul(e89[:, 256:384], kh[:, CK(9):CK(9) + 128],
                                 qh[:, CQ(8):CQ(8) + 128], start=True, stop=True)
                nc.scalar.activation(pt[:, 3072:3456], e89[:], AF.Exp)
                # edge masks: first/last local block of the batch row
                nc.gpsimd.tensor_mul(pt[:, 0:128], pt[:, 0:128],
                                     ml_sb[:].to_broadcast((128, 128)))
                nc.gpsimd.tensor_mul(pt[:, 3328:3456], pt[:, 3328:3456],
                                     mr_sb[:].to_broadcast((128, 128)))
                for tp in (2, 4, 6):
                    pp = pspr.tile([128, 1024], F32, tag="pspr", name="pp")
                    nc.tensor.matmul(pp[:, 0:384], kh[:, CK(tp):CK(tp) + 128],
                                     qh[:, CQ(tp - 1):CQ(tp - 1) + 384],
                                     start=True, stop=True)
                    # define the inter-bank gap bytes (read by the merged exp,
                    # never read from pt) with a same-stationary throwaway MM
                    nc.tensor.matmul(pp[:, 384:512], kh[:, CK(tp):CK(tp) + 128],
                                     qh[:, CQ(1):CQ(1) + 128],
                                     start=True, stop=True)
                    nc.tensor.matmul(psg[:, 2 * (tp - 1):2 * tp],
                                     kh[:, CK(tp):CK(tp) + 128], qg,
                                     start=False, stop=False)
                    nc.tensor.matmul(pp[:, 512:896],
                                     kh[:, CK(tp + 1):CK(tp + 1) + 128],
                                     qh[:, CQ(tp):CQ(tp) + 384],
                                     start=True, stop=True)
                    nc.tensor.matmul(psg[:, 2 * tp:2 * tp + 2],
                                     kh[:, CK(tp + 1):CK(tp + 1) + 128], qg,
                                     start=False, stop=(tp == 6))
                    nc.scalar.activation(
                        pt[:, PT_START[tp]:PT_START[tp] + 896],
                        pp[:, 0:896], AF.Exp)
                pg = psm.tile([128, 16], BF16, tag="pgsb", name="pg", bufs=3)
                nc.scalar.activation(pg[:], psg[:], AF.Exp)
                return {"pt": pt, "pxg2": pxg2, "pg": pg}

            def emit_tail(h, S, v96, vg_sb, psT):
                """Flipped P.V + normalize + transpose for head h."""
                hl = h % 8
                r0 = 64 * (hl % 2)
                r2 = 32 * (h % 2)
                pt, pxg2, pg = S["pt"], S["pxg2"], S["pg"]
                for c in range(2):
                    psA = pstl.tile([128, 260], F32, tag="pstl", name="psA")
                    psB = pstl.tile([128, 260], F32, tag="pstl", name="psB")
                    for j in range(4):
                        qs = 4 * c + j + 1
                        win = (qs - 1, qs, qs + 1)
                        for i, t in enumerate(win):
                            nc.tensor.matmul(
                                psA[:, 65 * j:65 * j + 65],
                                pt[:, ptcol(t, qs):ptcol(t, qs) + 128],
                                v96[t][:, 96 * hl:96 * hl + 65],
                                start=(i == 0), stop=(i == 2))
                        nc.tensor.matmul(
                            psB[:, 65 * j:65 * j + 65],
                            pxg2[r2:r2 + 2, 128 * (qs - 1):128 * qs],
                            vg_sb[r2:r2 + 2, 96 * hl:96 * hl + 65],
                            start=True, stop=True)
                    pA = psA.ap[0][0]
                    pB = psB.ap[0][0]
                    rA = psm.tile([128, 4], F32, tag="rA", name="rA")
                    rB = psm.tile([128, 4], F32, tag="rB", name="rB")
                    nc.vector.reciprocal(
                        rA[:], bass.AP(psA.tensor, psA[:].offset + 64,
                                       [[pA, 128], [65, 4]]))
                    nc.vector.reciprocal(
                        rB[:], bass.AP(psB.tensor, psB[:].offset + 64,
                                       [[pB, 128], [65, 4]]))
                    numA = bass.AP(psA.tensor, psA[:].offset,
                                   [[pA, 128], [65, 4], [1, 64]])
                    numB = bass.AP(psB.tensor, psB[:].offset,
                                   [[pB, 128], [65, 4], [1, 64]])
                    rAb = bass.AP(rA.tensor, rA[:].offset,
                                  [[rA.ap[0][0], 128], [1, 4], [0, 64]])
                    rBb = bass.AP(rB.tensor, rB[:].offset,
                                  [[rB.ap[0][0], 128], [1, 4], [0, 64]])
                    tmpA = psm.tile([128, 256], F32, tag="tmpA", name="tmpA")
                    tmpB = psm.tile([128, 256], F32, tag="tmpB", name="tmpB")
                    tA = bass.AP(tmpA.tensor, tmpA[:].offset,
                                 [[tmpA.ap[0][0], 128], [64, 4], [1, 64]])
                    tB = bass.AP(tmpB.tensor, tmpB[:].offset,
                                 [[tmpB.ap[0][0], 128], [64, 4], [1, 64]])
                    nc.vector.tensor_mul(tA, numA, rAb)
                    nc.vector.tensor_mul(tB, numB, rBb)
                    atq = psm.tile([128, 256], BF16, tag="atq", name="atq")
                    nc.gpsimd.tensor_add(atq[:], tmpA[:], tmpB[:])
                    for j in range(4):
                        nc.tensor.transpose(
                            psT[r0:r0 + 64,
                                512 * c + 128 * j:512 * c + 128 * j + 128],
                            atq[:, 64 * j:64 * j + 64], id_sb[:])
                # flash partials of the 2 global queries vs this core's keys
                ps_wv = pstl.tile([65, 2], F32, tag="pstl", name="ps_wv")
                for t in range(1, 9):
                    nc.tensor.matmul(ps_wv[:], v96[t][:, 96 * hl:96 * hl + 65],
                                     pg[:, 2 * (t - 1):2 * t],
                                     start=(t == 1), stop=(t == 8))
                nc.vector.tensor_copy(gst[:, 2 * h:2 * h + 2], ps_wv[:])
                if hl % 2 == 1:
                    for c in range(2):
                        nc.vector.tensor_copy(
                            at_sb[h // 2][:, 512 * c:512 * c + 512],
                            psT[:, 512 * c:512 * c + 512])

            # software-pipelined emission: scores of head h+1 are emitted
            # before the tail of head h; pass-B projections interleave at
            # group boundaries (PE executes in program order)
            qk0, qk1 = {}, {}
            emit_qk_proj(0, 0, qk0, pre=wts0)
            emit_qk_proj(0, 1, qk0)
            v96_0, vg0 = emit_v_proj(0)
            v96_1, vg1 = None, None
            S = {0: emit_scores(0, qk0)}
            psT = None
            pre10 = pre11 = None
            for h in range(16):
                if h + 1 < 16:
                    S[h + 1] = emit_scores(h + 1, qk0 if h + 1 < 8 else qk1)
                if h % 2 == 0:
                    psT = ppst.tile([128, 1024], BF16, tag="psT",
                                    name=f"psT{h}")
                v96, vg = (v96_0, vg0) if h < 8 else (v96_1, vg1)
                emit_tail(h, S.pop(h), v96, vg, psT)
                if h == 0:
                    pre10 = wqk_pre(1, 0)
                if h == 2:
                    emit_qk_proj(1, 0, qk1, pre=pre10)
                if h == 4:
                    pre11 = wqk_pre(1, 1)
                if h == 6:
                    emit_qk_proj(1, 1, qk1, pre=pre11)
                if h == 7:
                    v96_1, vg1 = emit_v_proj(1)

            # ================= output projection =================
            # flash partials for the host-side global rows can ship now
            nc.sync.dma_start(gstats, gst[:])
            # prefetch ALL weight tiles before the barrier so their DMAs
            # land during the attention tail
            wot_pre = []
            for m in range(8):
                wotp = pw.tile([128, 1024], BF16, tag="wo", bufs=8,
                               name=f"wot{m}")
                wsrc = bass.AP(wo.tensor, wo[m, 0].offset,
                               [[128, 128], [128 * 128, 8], [1, 128]])
                nc.sync.dma_start(wotp[:], wsrc)
                wot_pre.append(wotp)
            tc.no_sync_barrier()
            for m in range(8):
                # alternate PSUM pools and eviction engines across m so the
                # accumulate->bias->DMA chain of consecutive tiles pipelines
                if m % 2 == 0:
                    pp_op = pspr.tile([128, 1024], F32, tag="pspr", name="ppop")
                    ps_op = [pp_op[:, 0:512], pp_op[:, 512:1024]]
                else:
                    ps_op = [pstl.tile([128, 512], F32, tag="pstl",
                                       name=f"pop{c}")[:]
                             for c in range(2)]
                wot = wot_pre[m]
                for f in range(8):
                    for c in range(2):
                        nc.tensor.matmul(ps_op[c], wot[:, 128 * f:128 * f + 128],
                                         at_sb[f][:, 512 * c:512 * c + 512],
                                         start=(f == 0), stop=(f == 7))
                for c in range(2):
                    ot = pout.tile([128, 512], F32, tag="ot", bufs=4)
                    if c == 0:
                        nc.scalar.activation(ot[:], ps_op[c], AF.Identity,
                                             bias=bo_sb[:, m:m + 1])
                    else:
                        nc.vector.tensor_scalar_add(ot[:], ps_op[c],
                                                    bo_sb[:, m:m + 1])
                    nc.sync.dma_start(outt[128 * m:128 * (m + 1),
                                           512 * c:512 * c + 512], ot[:])
            if dbg is not None:
                for f in range(8):
                    nc.sync.dma_start(dbg[128 * f:128 * (f + 1), :], at_sb[f][:])
    return nc


_NC_CACHE = {}
LAST = {}


def get_nc():
    if "nc" not in _NC_CACHE:
        nc = bacc.Bacc("TRN2", target_bir_lowering=False, debug=False, num_devices=8)
        build_kernel(nc)
        nc.compile()
        _NC_CACHE["nc"] = nc
    return _NC_CACHE["nc"]


def make_inputs(x, Wq, Wk, Wv, Wo, bo):
    """Build the 8 per-core input maps (all host-side numpy)."""
    x = np.asarray(x, np.float32)
    Wq = np.asarray(Wq, np.float32)
    Wk = np.asarray(Wk, np.float32)
    Wv = np.asarray(Wv, np.float32)
    Wo = np.asarray(Wo, np.float32)
    bo = np.asarray(bo, np.float32)

    wq_r = (Wq * SCALE).T.reshape(8, 128, 8, 128).transpose(2, 0, 1, 3)
    wk_r = Wk.T.reshape(8, 128, 8, 128).transpose(2, 0, 1, 3)  # [ft, d, 128d, 128f]
    # [pss, half, d, 128d, 512]: per (pass, half, d) the 512 cols are
    # [q ft0 | q ft1 | k ft0 | k ft1]; SCALE folded into q
    qp = wq_r.reshape(2, 2, 2, 8, 128, 128).transpose(0, 1, 3, 4, 2, 5)
    kp = wk_r.reshape(2, 2, 2, 8, 128, 128).transpose(0, 1, 3, 4, 2, 5)
    wqk_r = np.ascontiguousarray(np.concatenate(
        [qp.reshape(2, 2, 8, 128, 256), kp.reshape(2, 2, 8, 128, 256)],
        -1)).astype(BF)
    wv_r = np.ascontiguousarray(
        Wv.T.reshape(8, 128, 2, 512).transpose(2, 0, 1, 3)).astype(BF)
    wo_r = np.ascontiguousarray(
        Wo.T.reshape(8, 128, 8, 128).transpose(2, 0, 1, 3)).astype(BF)
    # wo_r[m, f, i, j] must be Wo[128m+j, 128f+i] = Wo.T[128f+i, 128m+j]

    ones = np.ones((128, 1), BF)
    zeros = np.zeros((128, 1), BF)
    ident = np.eye(128, dtype=BF)
    in_maps = []
    for core in range(8):
        b, j = divmod(core, 4)
        # x-slice columns: [L0..L7, halo-left, halo-right]; globals shipped
        # separately (qgin/kgin/vgin), exact from the host
        xs = np.zeros((TOKS, D_MODEL), np.float32)
        for w in range(NW):
            gb = 8 * j - 1 + w
            col = 1024 if w == 0 else (1152 if w == 9 else 128 * (w - 1))
            if 0 <= gb < NB:
                xs[col:col + 128] = x[b, 1 + 128 * gb:1 + 128 * (gb + 1)]
        xg = x[b, [0, T - 1], :]                     # [2, D]
        qg = SCALE * (xg @ Wq.T)                     # [2, 1024]
        kg = xg @ Wk.T
        vgb = xg @ Wv.T
        # [pss, feature-row-within-tile, 2*tile + g]
        qgi = np.zeros((2, 128, 8), np.float32)
        kgi = np.zeros((2, 128, 8), np.float32)
        vgi = np.zeros((2, 34, 768), np.float32)
        for p in range(2):
            for i in range(4):
                f0 = 512 * p + 128 * i
                qgi[p, :, 2 * i:2 * i + 2] = qg[:, f0:f0 + 128].T
                kgi[p, :, 2 * i:2 * i + 2] = kg[:, f0:f0 + 128].T
            for hl in range(8):
                vgi[p, 0:2, 96 * hl:96 * hl + 64] = \
                    vgb[:, 512 * p + 64 * hl:512 * p + 64 * hl + 64]
                vgi[p, 0:2, 96 * hl + 64] = 1.0
            vgi[p, 32:34] = vgi[p, 0:2]
        in_maps.append({
            "xt": np.ascontiguousarray(xs.T).astype(BF),
            "wqk": wqk_r, "wv": wv_r, "wo": wo_r, "bo": bo,
            "maskl": zeros if j == 0 else ones,
            "maskr": zeros if j == 3 else ones,
            "ident": ident,
            "vgin": vgi.astype(BF),
            "qgin": qgi.astype(BF),
            "kgin": kgi.astype(BF),
        })
    return in_maps


def assemble_output(results, x, Wq, Wk, Wv, Wo, bo):
    x = np.asarray(x, np.float32)
    out = np.empty((B, T, D_MODEL), np.float32)
    for core in range(8):
        b, j = divmod(core, 4)
        out[b, 1 + 1024 * j:1 + 1024 * (j + 1), :] = results[core]["outt"].T

    # global token rows, exact on host
    xg = x[:, [0, T - 1], :]                      # [B, 2, D]
    qg = (xg @ Wq.T).reshape(B, 2, H, DK) * SCALE  # [B, 2, H, DK]
    kg = (xg @ Wk.T).reshape(B, 2, H, DK)
    vg = (xg @ Wv.T).reshape(B, 2, H, DK)
    for b in range(B):
        se = np.zeros((H, 2))
        wvs = np.zeros((H, 2, DK))
        for j in range(4):
            g = results[4 * b + j]["gstats"]  # [65, 32]
            for h in range(H):
                for gi in range(2):
                    se[h, gi] += g[64, 2 * h + gi]
                    wvs[h, gi] += g[0:64, 2 * h + gi]
        # add the global-key terms: scores qg . kg
        sgg = np.einsum("ghd,fhd->hgf", qg[b], kg[b])  # [H, 2g(query), 2f(key)]
        egg = np.exp(sgg)
        num = wvs + np.einsum("hgf,fhd->hgd", egg, vg[b])
        den = se + egg.sum(-1)
        og = num / den[..., None]                  # [H, 2, DK]
        for gi, trow in ((0, 0), (1, T - 1)):
            row = og[:, gi, :].reshape(H * DK)
            out[b, trow] = row @ Wo.T + bo
    return out


def kernel(x, Wq, Wk, Wv, Wo, bo):
    nc = get_nc()
    in_maps = make_inputs(x, Wq, Wk, Wv, Wo, bo)
    res = run_bass_kernel_spmd(nc, in_maps, core_ids=list(range(8)))
    LAST["res"] = res
    results = [{k: np.asarray(v) for k, v in r.items()} for r in res.results]
    return assemble_output(results, x, Wq, Wk, Wv, Wo, bo)


# revision 35
# speedup vs baseline: 2.1082x; 1.0341x over previous
"""BigBird sparse attention kernel for 8 Trainium2 NeuronCores.

Sharding: token-parallel. B=2 batches x 4 chunks of 1024 local tokens each
-> 8 cores. Each core receives a transposed bf16 x-slice [D=1024, 1280]
whose columns are [8 local 128-token blocks, halo-left, halo-right]
(halos zero-padded outside [0, 32)). Global-token q/k/v rows are computed
exactly on the host and shipped as tiny side inputs. The core computes:
  - q/k projections in transposed layout [f, tok] (bf16 matmuls, SCALE
    folded into the q weights host-side)
  - v projection in [tok, f] layout with a ones-column per head
  - 3-block sliding-window attention: scores kept transposed [kt, q],
    exp'd to bf16 probabilities (middle k-blocks share 2-bank psum tiles
    so two blocks exp in one ACT instruction); the P.V matmul is FLIPPED
    (stationary = P block, moving = V||ones) so each 128-token q block
    lands in PSUM as [q, 64 v-cols + denominator] with the softmax
    denominator per-partition
  - attention of local tokens to the 2 global tokens (separate softmax,
    paired across heads via a zero-masked [128, 34] stationary) ->
    normalize both with per-partition reciprocals, combine, transpose
    back to [feat, tok] on the PE array
  - flash-style partial stats (sum-exp, weighted V) of the 2 global query
    tokens against the core's local keys -> combined on host
  - output projection + bias for its 1024 local tokens
Host assembles the 8 slices, and computes the 2 global output rows per
batch exactly in numpy from the shipped partials.
"""

import numpy as np
import ml_dtypes

import concourse.bass as bass
import concourse.mybir as mybir
import concourse.tile as tile
from concourse import bacc
from concourse.bass_utils import run_bass_kernel_spmd

F32 = mybir.dt.float32
BF16 = mybir.dt.bfloat16
AF = mybir.ActivationFunctionType
BF = ml_dtypes.bfloat16

D_MODEL = 1024
H = 16
DK = 64
BS = 128
B = 2
T = 4098
NB = 32            # global 128-blocks of local tokens
NW = 10            # window blocks per core (8 local + 2 halo)
TOKS = NW * BS     # x-slice columns: [L0..L7, halo-left, halo-right]
SCALE = 1.0 / np.sqrt(DK)

CHUNKS_Q = [(0, 512), (512, 512)]
CHUNKS_K = [(0, 512), (512, 512), (1024, 256)]

# pt region start per window block t; middle blocks are exp'd in pairs from
# 2-bank psum tiles, leaving 128-col junk gaps (cols 768.., 1664.., 2560..)
PT_START = [0, 128, 384, 768, 1152, 1536, 1920, 2304, 2688, 2944]
PT_COLS = 3072


def CK(t):
    # column of window block t in the x-slice (k/v side)
    return 1024 if t == 0 else (1152 if t == 9 else BS * (t - 1))


def CQ(qs):
    # column of local q block qs (1..8) in the x-slice
    return BS * (qs - 1)


def qlo(t):
    return max(t - 1, 1)


def ptcol(t, qs):
    # column of (window-block t, q window-position qs) in the pt tensor
    return PT_START[t] + 128 * (qs - qlo(t))


def build_kernel(nc):
    xt = nc.dram_tensor("xt", [D_MODEL, TOKS], BF16, kind="ExternalInput").ap()
    wqk = nc.dram_tensor("wqk", [2, 2, 8, 128, 512], BF16,
                         kind="ExternalInput").ap()
    wv = nc.dram_tensor("wv", [2, 8, 128, 512], BF16, kind="ExternalInput").ap()
    vgin = nc.dram_tensor("vgin", [2, 34, 768], BF16, kind="ExternalInput").ap()
    qgin = nc.dram_tensor("qgin", [2, 128, 8], BF16, kind="ExternalInput").ap()
    kgin = nc.dram_tensor("kgin", [2, 128, 8], BF16, kind="ExternalInput").ap()
    wo = nc.dram_tensor("wo", [8, 8, 128, 128], BF16, kind="ExternalInput").ap()
    bo = nc.dram_tensor("bo", [D_MODEL], F32, kind="ExternalInput").ap()
    maskl = nc.dram_tensor("maskl", [128, 1], BF16, kind="ExternalInput").ap()
    maskr = nc.dram_tensor("maskr", [128, 1], BF16, kind="ExternalInput").ap()
    ident = nc.dram_tensor("ident", [128, 128], BF16, kind="ExternalInput").ap()
    outt = nc.dram_tensor("outt", [D_MODEL, 1024], F32, kind="ExternalOutput").ap()
    gstats = nc.dram_tensor("gstats", [65, 32], F32, kind="ExternalOutput").ap()
    import os as _os
    dbg = None
    if _os.environ.get("BB_DEBUG"):
        dbg = nc.dram_tensor("dbg_at", [D_MODEL, 1024], BF16,
                             kind="ExternalOutput").ap()

    with tile.TileContext(nc) as tc:
        with (
            tc.tile_pool(name="pc", bufs=1) as pc,
            tc.tile_pool(name="px", bufs=1) as px,
            tc.tile_pool(name="pqk", bufs=1) as pqk,
            tc.tile_pool(name="pv", bufs=1) as pvp,
            tc.tile_pool(name="pwv", bufs=1) as pwv,
            tc.tile_pool(name="pw", bufs=6) as pw,
            tc.tile_pool(name="pat", bufs=1) as pat,
            tc.tile_pool(name="ppt", bufs=2) as ppt,
            tc.tile_pool(name="psm", bufs=2) as psm,
            tc.tile_pool(name="pout", bufs=2) as pout,
            # PSUM: 2x 2-bank pair slots + 1 psg bank + 2x 1-bank + 1 psT
            tc.tile_pool(name="pspr", bufs=2, space="PSUM") as pspr,
            tc.tile_pool(name="ppsg", bufs=1, space="PSUM") as ppsg,
            tc.tile_pool(name="pstl", bufs=2, space="PSUM") as pstl,
            tc.tile_pool(name="ppst", bufs=1, space="PSUM") as ppst,
        ):
            # ---- constants ----
            bo_sb = pc.tile([128, 8], F32, tag="bo")
            nc.sync.dma_start(bo_sb[:], bo.rearrange("(t p) -> p t", p=128))
            ml_sb = pc.tile([128, 1], BF16, tag="ml")
            mr_sb = pc.tile([128, 1], BF16, tag="mr")
            nc.sync.dma_start(ml_sb[:], maskl)
            nc.sync.dma_start(mr_sb[:], maskr)
            id_sb = pc.tile([128, 128], BF16, tag="ident")
            nc.sync.dma_start(id_sb[:], ident)
            gst = pc.tile([65, 32], F32, tag="gst")

            def wqk_dma_pair(pss, half, dp, name):
                # [128, 1024] tile covering d=2dp (cols 0:512) and d=2dp+1;
                # each 512-col block is [q ft0 | q ft1 | k ft0 | k ft1]
                wt = pw.tile([128, 1024], BF16, tag="w", bufs=8, name=name)
                src = bass.AP(wqk.tensor, wqk[pss, half, 2 * dp].offset,
                              [[512, 128], [128 * 512, 2], [1, 512]])
                nc.sync.dma_start(wt[:], src)
                return wt

            def wqk_pre(pss, half):
                return [wqk_dma_pair(pss, half, dp, f"wt{pss}{half}_{dp}")
                        for dp in range(4)]

            # ---- x slice, transposed, resident; first-pass q/k weights
            # interleave so the projection d-loop starts immediately ----
            xts, wts0 = [], []
            for dp in range(4):
                wts0.append(wqk_dma_pair(0, 0, dp, f"wt00_{dp}"))
                for d in (2 * dp, 2 * dp + 1):
                    xd = px.tile([128, TOKS], BF16, tag=f"xt{d}")
                    nc.sync.dma_start(xd[:], xt[128 * d:128 * (d + 1), :])
                    xts.append(xd)

            at_sb = [pat.tile([128, 1024], BF16, tag=f"at{f}", name=f"at{f}")
                     for f in range(8)]

            GQK = {}

            def emit_qk_proj(pss, half, qk_tiles, pre=None):
                # SCALE is folded into the q weights host-side
                wts = pre if pre is not None else wqk_pre(pss, half)
                if half == 0:
                    qg_sb = psm.tile([128, 8], BF16, tag="qg", name="qg_sb")
                    kg_sb = psm.tile([128, 8], BF16, tag="kg", name="kg_sb")
                    nc.sync.dma_start(qg_sb[:], qgin[pss])
                    nc.sync.dma_start(kg_sb[:], kgin[pss])
                    GQK[pss] = (qg_sb, kg_sb)
                for pi, pname in enumerate(("q", "k")):
                    chunks = CHUNKS_Q if pname == "q" else CHUNKS_K
                    osbs, psjs = [], []
                    for i2 in range(2):
                        i = 2 * half + i2
                        osb = pqk.tile([128, TOKS], BF16, tag=f"qk{pname}{i}",
                                       name=f"qk{pname}{i}")
                        qk_tiles[(pname, i)] = osb
                        osbs.append(osb)
                        # chunks 0+1 pack into one 2-bank pair tile; the k
                        # halo chunk gets a 1-bank tile
                        pj = [pspr.tile([128, 1024], F32, tag="pspr",
                                        name=f"pj{i2}")]
                        if pname == "k":
                            pj.append(pstl.tile([128, 256], F32, tag="pstl",
                                                name=f"pjh{i2}"))
                        psjs.append(pj)
                    for d in range(8):
                        wcol = 512 * (d % 2) + 256 * pi
                        for i2 in range(2):
                            for c, (c0, cn) in enumerate(chunks):
                                dst = (psjs[i2][0][:, c0:c0 + cn] if c < 2
                                       else psjs[i2][1][:, 0:cn])
                                nc.tensor.matmul(
                                    dst,
                                    wts[d // 2][:, wcol + 128 * i2:
                                                wcol + 128 * i2 + 128],
                                    xts[d][:, c0:c0 + cn],
                                    start=(d == 0), stop=(d == 7))
                    # evictions split across VEC (q) and ACT (k) so the next
                    # d-loop's PSUM slots free without queue backlog
                    for i2 in range(2):
                        if pname == "q":
                            nc.vector.tensor_copy(osbs[i2][:, 0:1024],
                                                  psjs[i2][0][:])
                        else:
                            nc.scalar.copy(osbs[i2][:, 0:1024], psjs[i2][0][:])
                            nc.scalar.copy(osbs[i2][:, 1024:1280],
                                           psjs[i2][1][:])
                return qk_tiles

            def emit_v_proj(pss):
                wv_sb = []
                for d in range(8):
                    wvd = pwv.tile([128, 512], BF16, tag=f"wv{d}", name=f"wv{d}")
                    nc.sync.dma_start(wvd[:], wv[pss, d])
                    wv_sb.append(wvd)
                v96 = []
                for tb in range(NW):
                    pv_ps = pstl.tile([128, 512], F32, tag="pstl", name="pv_ps")
                    for d in range(8):
                        nc.tensor.matmul(pv_ps[:], xts[d][:, CK(tb):CK(tb) + 128],
                                         wv_sb[d][:], start=(d == 0), stop=(d == 7))
                    vt = pvp.tile([128, 8 * 96], BF16, tag=f"v96_{tb}",
                                  name=f"v96_{tb}")
                    pstep = vt.ap[0][0]
                    dst = bass.AP(vt.tensor, vt[:].offset,
                                  [[pstep, 128], [96, 8], [1, 64]])
                    src = bass.AP(pv_ps.tensor, pv_ps[:].offset,
                                  [[pv_ps.ap[0][0], 128], [64, 8], [1, 64]])
                    nc.vector.tensor_copy(dst, src)
                    onesap = bass.AP(vt.tensor, vt[:].offset + 64,
                                     [[pstep, 128], [96, 8], [1, 1]])
                    nc.vector.memset(onesap, 1.0)
                    v96.append(vt)
                # vg (global-token V rows + ones col) comes exact from the
                # host, duplicated at partitions 0:2 and 32:34 to match the
                # paired-pxg stationary placement
                vg_sb = pvp.tile([34, 8 * 96], BF16, tag="vg", name="vg")
                nc.sync.dma_start(vg_sb[:], vgin[pss])
                return v96, vg_sb

            pxg_pairs = {}

            def pair_runs(pp):
                # both 384-col banks of a score-pair psum tile as one 2-run AP
                return bass.AP(pp.tensor, pp[:].offset,
                               [[pp.ap[0][0], 128], [512, 2], [1, 384]])

            def emit_scores_edges(h, qk_tiles):
                """Paired pxg, edge-block scores (both edges share one psum
                pair tile, exp'd by a single strided ACT), psg start."""
                hl = h % 8
                r0 = 64 * (hl % 2)
                i = hl // 2
                qt = qk_tiles[("q", i)]
                kt_ = qk_tiles[("k", i)]
                qh = qt[r0:r0 + 64, :]
                kh = kt_[r0:r0 + 64, :]
                qg_sb, kg_sb = GQK[h // 8]
                qg = qg_sb[r0:r0 + 64, 2 * i:2 * i + 2]

                if h % 2 == 0:
                    # local-q -> global-k scores for BOTH heads of the pair in
                    # one matmul per 512-token chunk: stationary [128, 34] with
                    # head-even kg in (rows 0:64, cols 0:2) and head-odd kg in
                    # (rows 64:128, cols 32:34); zeros elsewhere mask the
                    # cross-head terms.
                    kg2 = psm.tile([128, 34], BF16, tag="kg2", name="kg2")
                    nc.vector.memset(kg2[:], 0.0)
                    nc.vector.tensor_copy(kg2[0:64, 0:2],
                                          kg_sb[0:64, 2 * i:2 * i + 2])
                    nc.vector.tensor_copy(kg2[64:128, 32:34],
                                          kg_sb[64:128, 2 * i:2 * i + 2])
                    pxg2 = psm.tile([34, 1024], BF16, tag="pxg", name="pxg2",
                                    bufs=2)
                    for c in range(2):
                        ps_xg = pstl.tile([34, 512], F32, tag="pstl",
                                          name="ps_xg")
                        nc.tensor.matmul(ps_xg[:], kg2[:],
                                         qt[:, 512 * c:512 * c + 512],
                                         start=True, stop=True)
                        nc.scalar.activation(pxg2[:, 512 * c:512 * c + 512],
                                             ps_xg[:], AF.Exp)
                    pxg_pairs[h // 2] = pxg2

                psg = ppsg.tile([128, 16], F32, tag="psg", name="psg")
                pt = ppt.tile([128, PT_COLS], BF16, tag="pt", name="pt")
                ep = pspr.tile([128, 1024], F32, tag="pspr", name="ep")
                nc.tensor.matmul(ep[:, 0:128], kh[:, CK(0):CK(0) + 128],
                                 qh[:, CQ(1):CQ(1) + 128], start=True, stop=True)
                nc.tensor.matmul(ep[:, 128:384], kh[:, CK(1):CK(1) + 128],
                                 qh[:, CQ(1):CQ(1) + 256], start=True, stop=True)
                nc.tensor.matmul(psg[:, 0:2], kh[:, CK(1):CK(1) + 128],
                                 qg, start=True, stop=False)
                nc.tensor.matmul(ep[:, 512:768], kh[:, CK(8):CK(8) + 128],
                                 qh[:, CQ(7):CQ(7) + 256], start=True, stop=True)
                nc.tensor.matmul(psg[:, 14:16], kh[:, CK(8):CK(8) + 128],
                                 qg, start=False, stop=False)
                nc.tensor.matmul(ep[:, 768:896], kh[:, CK(9):CK(9) + 128],
                                 qh[:, CQ(8):CQ(8) + 128], start=True, stop=True)
                ptedges = bass.AP(pt.tensor, pt[:].offset,
                                  [[pt.ap[0][0], 128], [2688, 2], [1, 384]])
                nc.scalar.activation(ptedges, pair_runs(ep), AF.Exp)
                # edge masks: first/last local block of the batch row
                nc.gpsimd.tensor_mul(pt[:, 0:128], pt[:, 0:128],
                                     ml_sb[:].to_broadcast((128, 128)))
                nc.gpsimd.tensor_mul(pt[:, 2944:3072], pt[:, 2944:3072],
                                     mr_sb[:].to_broadcast((128, 128)))
                return {"pt": pt, "pxg2": pxg_pairs[h // 2], "psg": psg,
                        "qh": qh, "kh": kh, "qg": qg}

            def emit_scores_pair(S, tp):
                """One middle k-block pair: 2 score MMs + psg riders + one
                strided-input exp into the compact pt."""
                qh, kh, qg, psg, pt = S["qh"], S["kh"], S["qg"], S["psg"], S["pt"]
                pp = pspr.tile([128, 1024], F32, tag="pspr", name="pp")
                nc.tensor.matmul(pp[:, 0:384], kh[:, CK(tp):CK(tp) + 128],
                                 qh[:, CQ(tp - 1):CQ(tp - 1) + 384],
                                 start=True, stop=True)
                nc.tensor.matmul(psg[:, 2 * (tp - 1):2 * tp],
                                 kh[:, CK(tp):CK(tp) + 128], qg,
                                 start=False, stop=False)
                nc.tensor.matmul(pp[:, 512:896],
                                 kh[:, CK(tp + 1):CK(tp + 1) + 128],
                                 qh[:, CQ(tp):CQ(tp) + 384],
                                 start=True, stop=True)
                nc.tensor.matmul(psg[:, 2 * tp:2 * tp + 2],
                                 kh[:, CK(tp + 1):CK(tp + 1) + 128], qg,
                                 start=False, stop=(tp == 6))
                dst = bass.AP(pt.tensor, pt[:, PT_START[tp]:].offset,
                              [[pt.ap[0][0], 128], [384, 2], [1, 384]])
                nc.scalar.activation(dst, pair_runs(pp), AF.Exp)

            def emit_scores_pg(S):
                pg = psm.tile([128, 16], BF16, tag="pgsb", name="pg", bufs=3)
                nc.scalar.activation(pg[:], S["psg"][:], AF.Exp)
                S["pg"] = pg

            def tail_pv(h, S, c, v96, vg_sb):
                """Flipped P.V + global-token P.V for one 512-token super
                block of head h."""
                hl = h % 8
                r2 = 32 * (h % 2)
                pt, pxg2 = S["pt"], S["pxg2"]
                psA = pstl.tile([128, 260], F32, tag="pstl", name="psA")
                psB = pstl.tile([128, 260], F32, tag="pstl", name="psB")
                S[("ps", c)] = (psA, psB)
                for j in range(4):
                    qs = 4 * c + j + 1
                    win = (qs - 1, qs, qs + 1)
                    for i, t in enumerate(win):
                        nc.tensor.matmul(
                            psA[:, 65 * j:65 * j + 65],
                            pt[:, ptcol(t, qs):ptcol(t, qs) + 128],
                            v96[t][:, 96 * hl:96 * hl + 65],
                            start=(i == 0), stop=(i == 2))
                    nc.tensor.matmul(
                        psB[:, 65 * j:65 * j + 65],
                        pxg2[r2:r2 + 2, 128 * (qs - 1):128 * qs],
                        vg_sb[r2:r2 + 2, 96 * hl:96 * hl + 65],
                        start=True, stop=True)

            def tail_norm(h, S, c):
                """Per-partition normalize + combine for super block c."""
                psA, psB = S[("ps", c)]
                pA = psA.ap[0][0]
                pB = psB.ap[0][0]
                rA = psm.tile([128, 4], F32, tag="rA", name="rA")
                rB = psm.tile([128, 4], F32, tag="rB", name="rB")
                nc.vector.reciprocal(
                    rA[:], bass.AP(psA.tensor, psA[:].offset + 64,
                                   [[pA, 128], [65, 4]]))
                nc.vector.reciprocal(
                    rB[:], bass.AP(psB.tensor, psB[:].offset + 64,
                                   [[pB, 128], [65, 4]]))
                numA = bass.AP(psA.tensor, psA[:].offset,
                               [[pA, 128], [65, 4], [1, 64]])
                numB = bass.AP(psB.tensor, psB[:].offset,
                               [[pB, 128], [65, 4], [1, 64]])
                rAb = bass.AP(rA.tensor, rA[:].offset,
                              [[rA.ap[0][0], 128], [1, 4], [0, 64]])
                rBb = bass.AP(rB.tensor, rB[:].offset,
                              [[rB.ap[0][0], 128], [1, 4], [0, 64]])
                tmpA = psm.tile([128, 256], F32, tag="tmpA", name="tmpA")
                tmpB = psm.tile([128, 256], F32, tag="tmpB", name="tmpB")
                tA = bass.AP(tmpA.tensor, tmpA[:].offset,
                             [[tmpA.ap[0][0], 128], [64, 4], [1, 64]])
                tB = bass.AP(tmpB.tensor, tmpB[:].offset,
                             [[tmpB.ap[0][0], 128], [64, 4], [1, 64]])
                nc.vector.tensor_mul(tA, numA, rAb)
                nc.vector.tensor_mul(tB, numB, rBb)
                atq = psm.tile([128, 256], BF16, tag="atq", name="atq")
                nc.gpsimd.tensor_add(atq[:], tmpA[:], tmpB[:])
                S[("atq", c)] = atq

            def tail_transposes(h, S, c, psT):
                hl = h % 8
                r0 = 64 * (hl % 2)
                atq = S[("atq", c)]
                for j in range(4):
                    nc.tensor.transpose(
                        psT[r0:r0 + 64,
                            512 * c + 128 * j:512 * c + 128 * j + 128],
                        atq[:, 64 * j:64 * j + 64], id_sb[:])

            def tail_wv(h, S, v96):
                # flash partials of the 2 global queries vs this core's keys
                hl = h % 8
                pg = S["pg"]
                ps_wv = pstl.tile([65, 2], F32, tag="pstl", name="ps_wv")
                for t in range(1, 9):
                    nc.tensor.matmul(ps_wv[:], v96[t][:, 96 * hl:96 * hl + 65],
                                     pg[:, 2 * (t - 1):2 * t],
                                     start=(t == 1), stop=(t == 8))
                nc.vector.tensor_copy(gst[:, 2 * h:2 * h + 2], ps_wv[:])

            def emit_scores_full(h, qk_tiles):
                S = emit_scores_edges(h, qk_tiles)
                for tp in (2, 4, 6):
                    emit_scores_pair(S, tp)
                emit_scores_pg(S)
                return S

            # software-pipelined emission: the tail of head h interleaves
            # between the score-pair emissions of head h+1, so the PE always
            # has P.V work while ACT digests the exps; pass-B projections
            # interleave at group boundaries (PE executes in program order)
            qk0, qk1 = {}, {}
            emit_qk_proj(0, 0, qk0, pre=wts0)
            emit_qk_proj(0, 1, qk0)
            v96_0, vg0 = emit_v_proj(0)
            v96_1, vg1 = None, None
            S = {0: emit_scores_full(0, qk0)}
            psT = None
            pre10 = pre11 = None
            for h in range(16):
                Sn = None
                if h + 1 < 16:
                    Sn = emit_scores_edges(h + 1, qk0 if h + 1 < 8 else qk1)
                if h % 2 == 0:
                    psT = ppst.tile([128, 1024], BF16, tag="psT",
                                    name=f"psT{h}")
                v96, vg = (v96_0, vg0) if h < 8 else (v96_1, vg1)
                Sc = S.pop(h)
                tail_pv(h, Sc, 0, v96, vg)
                if Sn is not None:
                    emit_scores_pair(Sn, 2)
                tail_norm(h, Sc, 0)
                if Sn is not None:
                    emit_scores_pair(Sn, 4)
                tail_transposes(h, Sc, 0, psT)
                tail_pv(h, Sc, 1, v96, vg)
                if Sn is not None:
                    emit_scores_pair(Sn, 6)
                    emit_scores_pg(Sn)
                tail_norm(h, Sc, 1)
                tail_wv(h, Sc, v96)
                tail_transposes(h, Sc, 1, psT)
                if h % 2 == 1:
                    for c in range(2):
                        nc.vector.tensor_copy(
                            at_sb[h // 2][:, 512 * c:512 * c + 512],
                            psT[:, 512 * c:512 * c + 512])
                if Sn is not None:
                    S[h + 1] = Sn
                if h == 0:
                    pre10 = wqk_pre(1, 0)
                if h == 2:
                    emit_qk_proj(1, 0, qk1, pre=pre10)
                if h == 4:
                    pre11 = wqk_pre(1, 1)
                if h == 6:
                    emit_qk_proj(1, 1, qk1, pre=pre11)
                if h == 7:
                    v96_1, vg1 = emit_v_proj(1)

            # ================= output projection =================
            # flash partials for the host-side global rows can ship now
            nc.sync.dma_start(gstats, gst[:])
            # prefetch ALL weight tiles before the barrier so their DMAs
            # land during the attention tail
            wot_pre = []
            for m in range(8):
                wotp = pw.tile([128, 1024], BF16, tag="wo", bufs=8,
                               name=f"wot{m}")
                wsrc = bass.AP(wo.tensor, wo[m, 0].offset,
                               [[128, 128], [128 * 128, 8], [1, 128]])
                nc.sync.dma_start(wotp[:], wsrc)
                wot_pre.append(wotp)
            tc.no_sync_barrier()
            for m in range(8):
                # alternate PSUM pools and eviction engines across m so the
                # accumulate->bias->DMA chain of consecutive tiles pipelines
                if m % 2 == 0:
                    pp_op = pspr.tile([128, 1024], F32, tag="pspr", name="ppop")
                    ps_op = [pp_op[:, 0:512], pp_op[:, 512:1024]]
                else:
                    ps_op = [pstl.tile([128, 512], F32, tag="pstl",
                                       name=f"pop{c}")[:]
                             for c in range(2)]
                wot = wot_pre[m]
                for f in range(8):
                    for c in range(2):
                        nc.tensor.matmul(ps_op[c], wot[:, 128 * f:128 * f + 128],
                                         at_sb[f][:, 512 * c:512 * c + 512],
                                         start=(f == 0), stop=(f == 7))
                for c in range(2):
                    ot = pout.tile([128, 512], F32, tag="ot", bufs=4)
                    if c == 0:
                        nc.scalar.activation(ot[:], ps_op[c], AF.Identity,
                                             bias=bo_sb[:, m:m + 1])
                    else:
                        nc.vector.tensor_scalar_add(ot[:], ps_op[c],
                                                    bo_sb[:, m:m + 1])
                    nc.sync.dma_start(outt[128 * m:128 * (m + 1),
                                           512 * c:512 * c + 512], ot[:])
            if dbg is not None:
                for f in range(8):
                    nc.sync.dma_start(dbg[128 * f:128 * (f + 1), :], at_sb[f][:])
    return nc


_NC_CACHE = {}
LAST = {}


def get_nc():
    if "nc" not in _NC_CACHE:
        nc = bacc.Bacc("TRN2", target_bir_lowering=False, debug=False, num_devices=8)
        build_kernel(nc)
        nc.compile()
        _NC_CACHE["nc"] = nc
    return _NC_CACHE["nc"]


def make_inputs(x, Wq, Wk, Wv, Wo, bo):
    """Build the 8 per-core input maps (all host-side numpy)."""
    x = np.asarray(x, np.float32)
    Wq = np.asarray(Wq, np.float32)
    Wk = np.asarray(Wk, np.float32)
    Wv = np.asarray(Wv, np.float32)
    Wo = np.asarray(Wo, np.float32)
    bo = np.asarray(bo, np.float32)

    wq_r = (Wq * SCALE).T.reshape(8, 128, 8, 128).transpose(2, 0, 1, 3)
    wk_r = Wk.T.reshape(8, 128, 8, 128).transpose(2, 0, 1, 3)  # [ft, d, 128d, 128f]
    # [pss, half, d, 128d, 512]: per (pass, half, d) the 512 cols are
    # [q ft0 | q ft1 | k ft0 | k ft1]; SCALE folded into q
    qp = wq_r.reshape(2, 2, 2, 8, 128, 128).transpose(0, 1, 3, 4, 2, 5)
    kp = wk_r.reshape(2, 2, 2, 8, 128, 128).transpose(0, 1, 3, 4, 2, 5)
    wqk_r = np.ascontiguousarray(np.concatenate(
        [qp.reshape(2, 2, 8, 128, 256), kp.reshape(2, 2, 8, 128, 256)],
        -1)).astype(BF)
    wv_r = np.ascontiguousarray(
        Wv.T.reshape(8, 128, 2, 512).transpose(2, 0, 1, 3)).astype(BF)
    wo_r = np.ascontiguousarray(
        Wo.T.reshape(8, 128, 8, 128).transpose(2, 0, 1, 3)).astype(BF)
    # wo_r[m, f, i, j] must be Wo[128m+j, 128f+i] = Wo.T[128f+i, 128m+j]

    ones = np.ones((128, 1), BF)
    zeros = np.zeros((128, 1), BF)
    ident = np.eye(128, dtype=BF)
    in_maps = []
    for core in range(8):
        b, j = divmod(core, 4)
        # x-slice columns: [L0..L7, halo-left, halo-right]; globals shipped
        # separately (qgin/kgin/vgin), exact from the host
        xs = np.zeros((TOKS, D_MODEL), np.float32)
        for w in range(NW):
            gb = 8 * j - 1 + w
            col = 1024 if w == 0 else (1152 if w == 9 else 128 * (w - 1))
            if 0 <= gb < NB:
                xs[col:col + 128] = x[b, 1 + 128 * gb:1 + 128 * (gb + 1)]
        xg = x[b, [0, T - 1], :]                     # [2, D]
        qg = SCALE * (xg @ Wq.T)                     # [2, 1024]
        kg = xg @ Wk.T
        vgb = xg @ Wv.T
        # [pss, feature-row-within-tile, 2*tile + g]
        qgi = np.zeros((2, 128, 8), np.float32)
        kgi = np.zeros((2, 128, 8), np.float32)
        vgi = np.zeros((2, 34, 768), np.float32)
        for p in range(2):
            for i in range(4):
                f0 = 512 * p + 128 * i
                qgi[p, :, 2 * i:2 * i + 2] = qg[:, f0:f0 + 128].T
                kgi[p, :, 2 * i:2 * i + 2] = kg[:, f0:f0 + 128].T
            for hl in range(8):
                vgi[p, 0:2, 96 * hl:96 * hl + 64] = \
                    vgb[:, 512 * p + 64 * hl:512 * p + 64 * hl + 64]
                vgi[p, 0:2, 96 * hl + 64] = 1.0
            vgi[p, 32:34] = vgi[p, 0:2]
        in_maps.append({
            "xt": np.ascontiguousarray(xs.T).astype(BF),
            "wqk": wqk_r, "wv": wv_r, "wo": wo_r, "bo": bo,
            "maskl": zeros if j == 0 else ones,
            "maskr": zeros if j == 3 else ones,
            "ident": ident,
            "vgin": vgi.astype(BF),
            "qgin": qgi.astype(BF),
            "kgin": kgi.astype(BF),
        })
    return in_maps


def assemble_output(results, x, Wq, Wk, Wv, Wo, bo):
    x = np.asarray(x, np.float32)
    out = np.empty((B, T, D_MODEL), np.float32)
    for core in range(8):
        b, j = divmod(core, 4)
        out[b, 1 + 1024 * j:1 + 1024 * (j + 1), :] = results[core]["outt"].T

    # global token rows, exact on host
    xg = x[:, [0, T - 1], :]                      # [B, 2, D]
    qg = (xg @ Wq.T).reshape(B, 2, H, DK) * SCALE  # [B, 2, H, DK]
    kg = (xg @ Wk.T).reshape(B, 2, H, DK)
    vg = (xg @ Wv.T).reshape(B, 2, H, DK)
    for b in range(B):
        se = np.zeros((H, 2))
        wvs = np.zeros((H, 2, DK))
        for j in range(4):
            g = results[4 * b + j]["gstats"]  # [65, 32]
            for h in range(H):
                for gi in range(2):
                    se[h, gi] += g[64, 2 * h + gi]
                    wvs[h, gi] += g[0:64, 2 * h + gi]
        # add the global-key terms: scores qg . kg
        sgg = np.einsum("ghd,fhd->hgf", qg[b], kg[b])  # [H, 2g(query), 2f(key)]
        egg = np.exp(sgg)
        num = wvs + np.einsum("hgf,fhd->hgd", egg, vg[b])
        den = se + egg.sum(-1)
        og = num / den[..., None]                  # [H, 2, DK]
        for gi, trow in ((0, 0), (1, T - 1)):
            row = og[:, gi, :].reshape(H * DK)
            out[b, trow] = row @ Wo.T + bo
    return out


def kernel(x, Wq, Wk, Wv, Wo, bo):
    nc = get_nc()
    in_maps = make_inputs(x, Wq, Wk, Wv, Wo, bo)
    res = run_bass_kernel_spmd(nc, in_maps, core_ids=list(range(8)))
    LAST["res"] = res
    results = [{k: np.asarray(v) for k, v in r.items()} for r in res.results]
    return assemble_output(results, x, Wq, Wk, Wv, Wo, bo)


# revision 38
# speedup vs baseline: 2.1172x; 1.0043x over previous
"""BigBird sparse attention kernel for 8 Trainium2 NeuronCores.

Sharding: token-parallel. B=2 batches x 4 chunks of 1024 local tokens each
-> 8 cores. Each core receives a transposed bf16 x-slice [D=1024, 1280]
whose columns are [8 local 128-token blocks, halo-left, halo-right]
(halos zero-padded outside [0, 32)). Global-token q/k/v rows are computed
exactly on the host and shipped as tiny side inputs. The core computes:
  - q/k projections in transposed layout [f, tok] (bf16 matmuls, SCALE
    folded into the q weights host-side)
  - v projection in [tok, f] layout with a ones-column per head
  - 3-block sliding-window attention: scores kept transposed [kt, q],
    exp'd to bf16 probabilities (middle k-blocks share 2-bank psum tiles
    so two blocks exp in one ACT instruction); the P.V matmul is FLIPPED
    (stationary = P block, moving = V||ones) so each 128-token q block
    lands in PSUM as [q, 64 v-cols + denominator] with the softmax
    denominator per-partition
  - attention of local tokens to the 2 global tokens (separate softmax,
    paired across heads via a zero-masked [128, 34] stationary) ->
    normalize both with per-partition reciprocals, combine, transpose
    back to [feat, tok] on the PE array
  - flash-style partial stats (sum-exp, weighted V) of the 2 global query
    tokens against the core's local keys -> combined on host
  - output projection + bias for its 1024 local tokens
Host assembles the 8 slices, and computes the 2 global output rows per
batch exactly in numpy from the shipped partials.
"""

import numpy as np
import ml_dtypes

import concourse.bass as bass
import concourse.mybir as mybir
import concourse.tile as tile
from concourse import bacc
from concourse.bass_utils import run_bass_kernel_spmd

F32 = mybir.dt.float32
BF16 = mybir.dt.bfloat16
AF = mybir.ActivationFunctionType
BF = ml_dtypes.bfloat16

D_MODEL = 1024
H = 16
DK = 64
BS = 128
B = 2
T = 4098
NB = 32            # global 128-blocks of local tokens
NW = 10            # window blocks per core (8 local + 2 halo)
TOKS = NW * BS     # x-slice columns: [L0..L7, halo-left, halo-right]
SCALE = 1.0 / np.sqrt(DK)

CHUNKS_Q = [(0, 512), (512, 512)]
CHUNKS_K = [(0, 512), (512, 512), (1024, 256)]

# pt region start per window block t; middle blocks are exp'd in pairs from
# 2-bank psum tiles, leaving 128-col junk gaps (cols 768.., 1664.., 2560..)
PT_START = [0, 128, 384, 768, 1152, 1536, 1920, 2304, 2688, 2944]
PT_COLS = 3072


def CK(t):
    # column of window block t in the x-slice (k/v side)
    return 1024 if t == 0 else (1152 if t == 9 else BS * (t - 1))


def CQ(qs):
    # column of local q block qs (1..8) in the x-slice
    return BS * (qs - 1)


def qlo(t):
    return max(t - 1, 1)


def ptcol(t, qs):
    # column of (window-block t, q window-position qs) in the pt tensor
    return PT_START[t] + 128 * (qs - qlo(t))


def build_kernel(nc):
    xt = nc.dram_tensor("xt", [D_MODEL, TOKS], BF16, kind="ExternalInput").ap()
    wqk = nc.dram_tensor("wqk", [2, 2, 8, 128, 512], BF16,
                         kind="ExternalInput").ap()
    wv = nc.dram_tensor("wv", [2, 8, 128, 512], BF16, kind="ExternalInput").ap()
    vgin = nc.dram_tensor("vgin", [2, 34, 768], BF16, kind="ExternalInput").ap()
    qgin = nc.dram_tensor("qgin", [2, 128, 8], BF16, kind="ExternalInput").ap()
    kgin = nc.dram_tensor("kgin", [2, 128, 8], BF16, kind="ExternalInput").ap()
    wo = nc.dram_tensor("wo", [8, 8, 128, 128], BF16, kind="ExternalInput").ap()
    bo = nc.dram_tensor("bo", [D_MODEL], F32, kind="ExternalInput").ap()
    maskl = nc.dram_tensor("maskl", [128, 1], BF16, kind="ExternalInput").ap()
    maskr = nc.dram_tensor("maskr", [128, 1], BF16, kind="ExternalInput").ap()
    ident = nc.dram_tensor("ident", [128, 128], BF16, kind="ExternalInput").ap()
    outt = nc.dram_tensor("outt", [D_MODEL, 1024], F32, kind="ExternalOutput").ap()
    gstats = nc.dram_tensor("gstats", [65, 32], F32, kind="ExternalOutput").ap()
    import os as _os
    dbg = None
    if _os.environ.get("BB_DEBUG"):
        dbg = nc.dram_tensor("dbg_at", [D_MODEL, 1024], BF16,
                             kind="ExternalOutput").ap()

    with tile.TileContext(nc) as tc:
        with (
            tc.tile_pool(name="pc", bufs=1) as pc,
            tc.tile_pool(name="px", bufs=1) as px,
            tc.tile_pool(name="pqk", bufs=1) as pqk,
            tc.tile_pool(name="pv", bufs=1) as pvp,
            tc.tile_pool(name="pwv", bufs=1) as pwv,
            tc.tile_pool(name="pw", bufs=6) as pw,
            tc.tile_pool(name="pat", bufs=1) as pat,
            tc.tile_pool(name="ppt", bufs=2) as ppt,
            tc.tile_pool(name="psm", bufs=2) as psm,
            tc.tile_pool(name="pout", bufs=2) as pout,
            # PSUM: 2x 2-bank pair slots + 1 psg bank + 2x 1-bank + 1 psT
            tc.tile_pool(name="pspr", bufs=2, space="PSUM") as pspr,
            tc.tile_pool(name="ppsg", bufs=1, space="PSUM") as ppsg,
            tc.tile_pool(name="pstl", bufs=2, space="PSUM") as pstl,
            tc.tile_pool(name="ppst", bufs=1, space="PSUM") as ppst,
        ):
            gst = pc.tile([65, 32], F32, tag="gst")

            def wqk_dma_pair(pss, half, dp, name):
                # [128, 1024] tile covering d=2dp (cols 0:512) and d=2dp+1;
                # each 512-col block is [q ft0 | q ft1 | k ft0 | k ft1]
                wt = pw.tile([128, 1024], BF16, tag="w", bufs=8, name=name)
                src = bass.AP(wqk.tensor, wqk[pss, half, 2 * dp].offset,
                              [[512, 128], [128 * 512, 2], [1, 512]])
                nc.sync.dma_start(wt[:], src)
                return wt

            def wqk_pre(pss, half):
                return [wqk_dma_pair(pss, half, dp, f"wt{pss}{half}_{dp}")
                        for dp in range(4)]

            # ---- x slice, transposed, resident; first-pass q/k weights
            # interleave so the projection d-loop starts immediately ----
            xts, wts0 = [], []
            for dp in range(4):
                wts0.append(wqk_dma_pair(0, 0, dp, f"wt00_{dp}"))
                for d in (2 * dp, 2 * dp + 1):
                    xd = px.tile([128, TOKS], BF16, tag=f"xt{d}")
                    nc.sync.dma_start(xd[:], xt[128 * d:128 * (d + 1), :])
                    xts.append(xd)
                if dp == 0:
                    # constants ride behind the first weight/x tiles so they
                    # don't delay the projection d-loop start
                    bo_sb = pc.tile([128, 8], F32, tag="bo")
                    nc.sync.dma_start(bo_sb[:],
                                      bo.rearrange("(t p) -> p t", p=128))
                    ml_sb = pc.tile([128, 1], BF16, tag="ml")
                    mr_sb = pc.tile([128, 1], BF16, tag="mr")
                    nc.sync.dma_start(ml_sb[:], maskl)
                    nc.sync.dma_start(mr_sb[:], maskr)
                    id_sb = pc.tile([128, 128], BF16, tag="ident")
                    nc.sync.dma_start(id_sb[:], ident)

            at_sb = [pat.tile([128, 1024], BF16, tag=f"at{f}", name=f"at{f}")
                     for f in range(8)]

            GQK = {}

            def emit_qk_proj(pss, half, qk_tiles, pre=None):
                # SCALE is folded into the q weights host-side
                wts = pre if pre is not None else wqk_pre(pss, half)
                if half == 0:
                    qg_sb = psm.tile([128, 8], BF16, tag="qg", name="qg_sb")
                    kg_sb = psm.tile([128, 8], BF16, tag="kg", name="kg_sb")
                    nc.sync.dma_start(qg_sb[:], qgin[pss])
                    nc.sync.dma_start(kg_sb[:], kgin[pss])
                    GQK[pss] = (qg_sb, kg_sb)
                for pi, pname in enumerate(("q", "k")):
                    chunks = CHUNKS_Q if pname == "q" else CHUNKS_K
                    osbs, psjs = [], []
                    for i2 in range(2):
                        i = 2 * half + i2
                        osb = pqk.tile([128, TOKS], BF16, tag=f"qk{pname}{i}",
                                       name=f"qk{pname}{i}")
                        qk_tiles[(pname, i)] = osb
                        osbs.append(osb)
                        # chunks 0+1 pack into one 2-bank pair tile; the k
                        # halo chunk gets a 1-bank tile
                        pj = [pspr.tile([128, 1024], F32, tag="pspr",
                                        name=f"pj{i2}")]
                        if pname == "k":
                            pj.append(pstl.tile([128, 256], F32, tag="pstl",
                                                name=f"pjh{i2}"))
                        psjs.append(pj)
                    for d in range(8):
                        wcol = 512 * (d % 2) + 256 * pi
                        for i2 in range(2):
                            for c, (c0, cn) in enumerate(chunks):
                                dst = (psjs[i2][0][:, c0:c0 + cn] if c < 2
                                       else psjs[i2][1][:, 0:cn])
                                nc.tensor.matmul(
                                    dst,
                                    wts[d // 2][:, wcol + 128 * i2:
                                                wcol + 128 * i2 + 128],
                                    xts[d][:, c0:c0 + cn],
                                    start=(d == 0), stop=(d == 7))
                    # evictions split across VEC (q) and ACT (k) so the next
                    # d-loop's PSUM slots free without queue backlog
                    for i2 in range(2):
                        if pname == "q":
                            nc.vector.tensor_copy(osbs[i2][:, 0:1024],
                                                  psjs[i2][0][:])
                        else:
                            nc.scalar.copy(osbs[i2][:, 0:1024], psjs[i2][0][:])
                            nc.scalar.copy(osbs[i2][:, 1024:1280],
                                           psjs[i2][1][:])
                return qk_tiles

            def emit_v_proj(pss):
                wv_sb = []
                for d in range(8):
                    wvd = pwv.tile([128, 512], BF16, tag=f"wv{d}", name=f"wv{d}")
                    nc.sync.dma_start(wvd[:], wv[pss, d])
                    wv_sb.append(wvd)
                v96 = []
                for tb in range(NW):
                    pv_ps = pstl.tile([128, 512], F32, tag="pstl", name="pv_ps")
                    for d in range(8):
                        nc.tensor.matmul(pv_ps[:], xts[d][:, CK(tb):CK(tb) + 128],
                                         wv_sb[d][:], start=(d == 0), stop=(d == 7))
                    vt = pvp.tile([128, 8 * 96], BF16, tag=f"v96_{tb}",
                                  name=f"v96_{tb}")
                    pstep = vt.ap[0][0]
                    dst = bass.AP(vt.tensor, vt[:].offset,
                                  [[pstep, 128], [96, 8], [1, 64]])
                    src = bass.AP(pv_ps.tensor, pv_ps[:].offset,
                                  [[pv_ps.ap[0][0], 128], [64, 8], [1, 64]])
                    nc.vector.tensor_copy(dst, src)
                    onesap = bass.AP(vt.tensor, vt[:].offset + 64,
                                     [[pstep, 128], [96, 8], [1, 1]])
                    nc.vector.memset(onesap, 1.0)
                    v96.append(vt)
                # vg (global-token V rows + ones col) comes exact from the
                # host, duplicated at partitions 0:2 and 32:34 to match the
                # paired-pxg stationary placement
                vg_sb = pvp.tile([34, 8 * 96], BF16, tag="vg", name="vg")
                nc.sync.dma_start(vg_sb[:], vgin[pss])
                return v96, vg_sb

            pxg_pairs = {}

            def pair_runs(pp):
                # both 384-col banks of a score-pair psum tile as one 2-run AP
                return bass.AP(pp.tensor, pp[:].offset,
                               [[pp.ap[0][0], 128], [512, 2], [1, 384]])

            def emit_scores_edges(h, qk_tiles):
                """Paired pxg, edge-block scores (both edges share one psum
                pair tile, exp'd by a single strided ACT), psg start."""
                hl = h % 8
                r0 = 64 * (hl % 2)
                i = hl // 2
                qt = qk_tiles[("q", i)]
                kt_ = qk_tiles[("k", i)]
                qh = qt[r0:r0 + 64, :]
                kh = kt_[r0:r0 + 64, :]
                qg_sb, kg_sb = GQK[h // 8]
                qg = qg_sb[r0:r0 + 64, 2 * i:2 * i + 2]

                if h % 2 == 0:
                    # local-q -> global-k scores for BOTH heads of the pair in
                    # one matmul per 512-token chunk: stationary [128, 34] with
                    # head-even kg in (rows 0:64, cols 0:2) and head-odd kg in
                    # (rows 64:128, cols 32:34); zeros elsewhere mask the
                    # cross-head terms.
                    kg2 = psm.tile([128, 34], BF16, tag="kg2", name="kg2")
                    nc.vector.memset(kg2[:], 0.0)
                    nc.vector.tensor_copy(kg2[0:64, 0:2],
                                          kg_sb[0:64, 2 * i:2 * i + 2])
                    nc.vector.tensor_copy(kg2[64:128, 32:34],
                                          kg_sb[64:128, 2 * i:2 * i + 2])
                    pxg2 = psm.tile([34, 1024], BF16, tag="pxg", name="pxg2",
                                    bufs=2)
                    for c in range(2):
                        ps_xg = pstl.tile([34, 512], F32, tag="pstl",
                                          name="ps_xg")
                        nc.tensor.matmul(ps_xg[:], kg2[:],
                                         qt[:, 512 * c:512 * c + 512],
                                         start=True, stop=True)
                        nc.scalar.activation(pxg2[:, 512 * c:512 * c + 512],
                                             ps_xg[:], AF.Exp)
                    pxg_pairs[h // 2] = pxg2

                psg = ppsg.tile([128, 16], F32, tag="psg", name="psg")
                pt = ppt.tile([128, PT_COLS], BF16, tag="pt", name="pt")
                ep = pspr.tile([128, 1024], F32, tag="pspr", name="ep")
                nc.tensor.matmul(ep[:, 0:128], kh[:, CK(0):CK(0) + 128],
                                 qh[:, CQ(1):CQ(1) + 128], start=True, stop=True)
                nc.tensor.matmul(ep[:, 128:384], kh[:, CK(1):CK(1) + 128],
                                 qh[:, CQ(1):CQ(1) + 256], start=True, stop=True)
                nc.tensor.matmul(psg[:, 0:2], kh[:, CK(1):CK(1) + 128],
                                 qg, start=True, stop=False)
                nc.tensor.matmul(ep[:, 512:768], kh[:, CK(8):CK(8) + 128],
                                 qh[:, CQ(7):CQ(7) + 256], start=True, stop=True)
                nc.tensor.matmul(psg[:, 14:16], kh[:, CK(8):CK(8) + 128],
                                 qg, start=False, stop=False)
                nc.tensor.matmul(ep[:, 768:896], kh[:, CK(9):CK(9) + 128],
                                 qh[:, CQ(8):CQ(8) + 128], start=True, stop=True)
                ptedges = bass.AP(pt.tensor, pt[:].offset,
                                  [[pt.ap[0][0], 128], [2688, 2], [1, 384]])
                nc.scalar.activation(ptedges, pair_runs(ep), AF.Exp)
                # edge masks: first/last local block of the batch row
                nc.gpsimd.tensor_mul(pt[:, 0:128], pt[:, 0:128],
                                     ml_sb[:].to_broadcast((128, 128)))
                nc.gpsimd.tensor_mul(pt[:, 2944:3072], pt[:, 2944:3072],
                                     mr_sb[:].to_broadcast((128, 128)))
                return {"pt": pt, "pxg2": pxg_pairs[h // 2], "psg": psg,
                        "qh": qh, "kh": kh, "qg": qg}

            def emit_scores_pair(S, tp):
                """One middle k-block pair: 2 score MMs + psg riders + one
                strided-input exp into the compact pt."""
                qh, kh, qg, psg, pt = S["qh"], S["kh"], S["qg"], S["psg"], S["pt"]
                pp = pspr.tile([128, 1024], F32, tag="pspr", name="pp")
                nc.tensor.matmul(pp[:, 0:384], kh[:, CK(tp):CK(tp) + 128],
                                 qh[:, CQ(tp - 1):CQ(tp - 1) + 384],
                                 start=True, stop=True)
                nc.tensor.matmul(psg[:, 2 * (tp - 1):2 * tp],
                                 kh[:, CK(tp):CK(tp) + 128], qg,
                                 start=False, stop=False)
                nc.tensor.matmul(pp[:, 512:896],
                                 kh[:, CK(tp + 1):CK(tp + 1) + 128],
                                 qh[:, CQ(tp):CQ(tp) + 384],
                                 start=True, stop=True)
                nc.tensor.matmul(psg[:, 2 * tp:2 * tp + 2],
                                 kh[:, CK(tp + 1):CK(tp + 1) + 128], qg,
                                 start=False, stop=(tp == 6))
                dst = bass.AP(pt.tensor, pt[:, PT_START[tp]:].offset,
                              [[pt.ap[0][0], 128], [384, 2], [1, 384]])
                nc.scalar.activation(dst, pair_runs(pp), AF.Exp)

            def emit_scores_pg(S):
                pg = psm.tile([128, 16], BF16, tag="pgsb", name="pg", bufs=3)
                nc.scalar.activation(pg[:], S["psg"][:], AF.Exp)
                S["pg"] = pg

            def tail_pv(h, S, c, v96, vg_sb):
                """Flipped P.V + global-token P.V for one 512-token super
                block of head h."""
                hl = h % 8
                r2 = 32 * (h % 2)
                pt, pxg2 = S["pt"], S["pxg2"]
                psA = pstl.tile([128, 260], F32, tag="pstl", name="psA")
                psB = pstl.tile([128, 260], F32, tag="pstl", name="psB")
                S[("ps", c)] = (psA, psB)
                for j in range(4):
                    qs = 4 * c + j + 1
                    win = (qs - 1, qs, qs + 1)
                    for i, t in enumerate(win):
                        nc.tensor.matmul(
                            psA[:, 65 * j:65 * j + 65],
                            pt[:, ptcol(t, qs):ptcol(t, qs) + 128],
                            v96[t][:, 96 * hl:96 * hl + 65],
                            start=(i == 0), stop=(i == 2))
                    nc.tensor.matmul(
                        psB[:, 65 * j:65 * j + 65],
                        pxg2[r2:r2 + 2, 128 * (qs - 1):128 * qs],
                        vg_sb[r2:r2 + 2, 96 * hl:96 * hl + 65],
                        start=True, stop=True)

            def tail_norm(h, S, c):
                """Per-partition normalize + combine for super block c."""
                psA, psB = S[("ps", c)]
                pA = psA.ap[0][0]
                pB = psB.ap[0][0]
                rA = psm.tile([128, 4], F32, tag="rA", name="rA")
                rB = psm.tile([128, 4], F32, tag="rB", name="rB")
                nc.vector.reciprocal(
                    rA[:], bass.AP(psA.tensor, psA[:].offset + 64,
                                   [[pA, 128], [65, 4]]))
                nc.vector.reciprocal(
                    rB[:], bass.AP(psB.tensor, psB[:].offset + 64,
                                   [[pB, 128], [65, 4]]))
                numA = bass.AP(psA.tensor, psA[:].offset,
                               [[pA, 128], [65, 4], [1, 64]])
                numB = bass.AP(psB.tensor, psB[:].offset,
                               [[pB, 128], [65, 4], [1, 64]])
                rAb = bass.AP(rA.tensor, rA[:].offset,
                              [[rA.ap[0][0], 128], [1, 4], [0, 64]])
                rBb = bass.AP(rB.tensor, rB[:].offset,
                              [[rB.ap[0][0], 128], [1, 4], [0, 64]])
                tmpA = psm.tile([128, 256], F32, tag="tmpA", name="tmpA")
                tmpB = psm.tile([128, 256], F32, tag="tmpB", name="tmpB")
                tA = bass.AP(tmpA.tensor, tmpA[:].offset,
                             [[tmpA.ap[0][0], 128], [64, 4], [1, 64]])
                tB = bass.AP(tmpB.tensor, tmpB[:].offset,
                             [[tmpB.ap[0][0], 128], [64, 4], [1, 64]])
                nc.vector.tensor_mul(tA, numA, rAb)
                nc.vector.tensor_mul(tB, numB, rBb)
                atq = psm.tile([128, 256], BF16, tag="atq", name="atq")
                nc.gpsimd.tensor_add(atq[:], tmpA[:], tmpB[:])
                S[("atq", c)] = atq

            def tail_transposes(h, S, c, psT):
                hl = h % 8
                r0 = 64 * (hl % 2)
                atq = S[("atq", c)]
                for j in range(4):
                    nc.tensor.transpose(
                        psT[r0:r0 + 64,
                            512 * c + 128 * j:512 * c + 128 * j + 128],
                        atq[:, 64 * j:64 * j + 64], id_sb[:])

            def tail_wv(h, S, v96):
                # flash partials of the 2 global queries vs this core's keys
                hl = h % 8
                pg = S["pg"]
                ps_wv = pstl.tile([65, 2], F32, tag="pstl", name="ps_wv")
                for t in range(1, 9):
                    nc.tensor.matmul(ps_wv[:], v96[t][:, 96 * hl:96 * hl + 65],
                                     pg[:, 2 * (t - 1):2 * t],
                                     start=(t == 1), stop=(t == 8))
                nc.vector.tensor_copy(gst[:, 2 * h:2 * h + 2], ps_wv[:])

            def emit_scores_full(h, qk_tiles):
                S = emit_scores_edges(h, qk_tiles)
                for tp in (2, 4, 6):
                    emit_scores_pair(S, tp)
                emit_scores_pg(S)
                return S

            # software-pipelined emission: the tail of head h interleaves
            # between the score-pair emissions of head h+1, so the PE always
            # has P.V work while ACT digests the exps; pass-B projections
            # interleave at group boundaries (PE executes in program order)
            qk0, qk1 = {}, {}
            emit_qk_proj(0, 0, qk0, pre=wts0)
            emit_qk_proj(0, 1, qk0)
            v96_0, vg0 = emit_v_proj(0)
            v96_1, vg1 = None, None
            S = {0: emit_scores_full(0, qk0)}
            psT = None
            pre10 = pre11 = None
            for h in range(16):
                Sn = None
                if h + 1 < 16:
                    Sn = emit_scores_edges(h + 1, qk0 if h + 1 < 8 else qk1)
                if h % 2 == 0:
                    psT = ppst.tile([128, 1024], BF16, tag="psT",
                                    name=f"psT{h}")
                v96, vg = (v96_0, vg0) if h < 8 else (v96_1, vg1)
                Sc = S.pop(h)
                tail_pv(h, Sc, 0, v96, vg)
                if Sn is not None:
                    emit_scores_pair(Sn, 2)
                tail_norm(h, Sc, 0)
                if Sn is not None:
                    emit_scores_pair(Sn, 4)
                tail_transposes(h, Sc, 0, psT)
                tail_pv(h, Sc, 1, v96, vg)
                if Sn is not None:
                    emit_scores_pair(Sn, 6)
                    emit_scores_pg(Sn)
                tail_norm(h, Sc, 1)
                tail_wv(h, Sc, v96)
                tail_transposes(h, Sc, 1, psT)
                if h % 2 == 1:
                    for c in range(2):
                        nc.vector.tensor_copy(
                            at_sb[h // 2][:, 512 * c:512 * c + 512],
                            psT[:, 512 * c:512 * c + 512])
                if Sn is not None:
                    S[h + 1] = Sn
                if h == 0:
                    pre10 = wqk_pre(1, 0)
                if h == 2:
                    emit_qk_proj(1, 0, qk1, pre=pre10)
                if h == 4:
                    pre11 = wqk_pre(1, 1)
                if h == 6:
                    emit_qk_proj(1, 1, qk1, pre=pre11)
                if h == 7:
                    v96_1, vg1 = emit_v_proj(1)

            # ================= output projection =================
            # flash partials for the host-side global rows can ship now
            nc.sync.dma_start(gstats, gst[:])
            # prefetch ALL weight tiles before the barrier so their DMAs
            # land during the attention tail
            wot_pre = []
            for m in range(8):
                wotp = pw.tile([128, 1024], BF16, tag="wo", bufs=8,
                               name=f"wot{m}")
                wsrc = bass.AP(wo.tensor, wo[m, 0].offset,
                               [[128, 128], [128 * 128, 8], [1, 128]])
                nc.sync.dma_start(wotp[:], wsrc)
                wot_pre.append(wotp)
            tc.no_sync_barrier()
            for m in range(8):
                # rotate PSUM pools 4-deep across m (the psg/psT banks are
                # free after the attention phase) and alternate eviction
                # engines so the accumulate->bias->DMA chains pipeline
                if m % 4 == 3:
                    ps_op = [ppsg.tile([128, 512], F32, tag="psg",
                                       name="popg")[:],
                             ppst.tile([128, 512], F32, tag="psT",
                                       name="popt")[:]]
                elif m % 2 == 0:
                    pp_op = pspr.tile([128, 1024], F32, tag="pspr", name="ppop")
                    ps_op = [pp_op[:, 0:512], pp_op[:, 512:1024]]
                else:
                    ps_op = [pstl.tile([128, 512], F32, tag="pstl",
                                       name=f"pop{c}")[:]
                             for c in range(2)]
                wot = wot_pre[m]
                for f in range(8):
                    for c in range(2):
                        nc.tensor.matmul(ps_op[c], wot[:, 128 * f:128 * f + 128],
                                         at_sb[f][:, 512 * c:512 * c + 512],
                                         start=(f == 0), stop=(f == 7))
                for c in range(2):
                    ot = pout.tile([128, 512], F32, tag="ot", bufs=4)
                    if c == 0:
                        nc.scalar.activation(ot[:], ps_op[c], AF.Identity,
                                             bias=bo_sb[:, m:m + 1])
                    else:
                        nc.vector.tensor_scalar_add(ot[:], ps_op[c],
                                                    bo_sb[:, m:m + 1])
                    nc.sync.dma_start(outt[128 * m:128 * (m + 1),
                                           512 * c:512 * c + 512], ot[:])
            if dbg is not None:
                for f in range(8):
                    nc.sync.dma_start(dbg[128 * f:128 * (f + 1), :], at_sb[f][:])
    return nc


_NC_CACHE = {}
LAST = {}


def get_nc():
    if "nc" not in _NC_CACHE:
        nc = bacc.Bacc("TRN2", target_bir_lowering=False, debug=False, num_devices=8)
        build_kernel(nc)
        nc.compile()
        _NC_CACHE["nc"] = nc
    return _NC_CACHE["nc"]


def make_inputs(x, Wq, Wk, Wv, Wo, bo):
    """Build the 8 per-core input maps (all host-side numpy)."""
    x = np.asarray(x, np.float32)
    Wq = np.asarray(Wq, np.float32)
    Wk = np.asarray(Wk, np.float32)
    Wv = np.asarray(Wv, np.float32)
    Wo = np.asarray(Wo, np.float32)
    bo = np.asarray(bo, np.float32)

    wq_r = (Wq * SCALE).T.reshape(8, 128, 8, 128).transpose(2, 0, 1, 3)
    wk_r = Wk.T.reshape(8, 128, 8, 128).transpose(2, 0, 1, 3)  # [ft, d, 128d, 128f]
    # [pss, half, d, 128d, 512]: per (pass, half, d) the 512 cols are
    # [q ft0 | q ft1 | k ft0 | k ft1]; SCALE folded into q
    qp = wq_r.reshape(2, 2, 2, 8, 128, 128).transpose(0, 1, 3, 4, 2, 5)
    kp = wk_r.reshape(2, 2, 2, 8, 128, 128).transpose(0, 1, 3, 4, 2, 5)
    wqk_r = np.ascontiguousarray(np.concatenate(
        [qp.reshape(2, 2, 8, 128, 256), kp.reshape(2, 2, 8, 128, 256)],
        -1)).astype(BF)
    wv_r = np.ascontiguousarray(
        Wv.T.reshape(8, 128, 2, 512).transpose(2, 0, 1, 3)).astype(BF)
    wo_r = np.ascontiguousarray(
        Wo.T.reshape(8, 128, 8, 128).transpose(2, 0, 1, 3)).astype(BF)
    # wo_r[m, f, i, j] must be Wo[128m+j, 128f+i] = Wo.T[128f+i, 128m+j]

    ones = np.ones((128, 1), BF)
    zeros = np.zeros((128, 1), BF)
    ident = np.eye(128, dtype=BF)
    in_maps = []
    for core in range(8):
        b, j = divmod(core, 4)
        # x-slice columns: [L0..L7, halo-left, halo-right]; globals shipped
        # separately (qgin/kgin/vgin), exact from the host
        xs = np.zeros((TOKS, D_MODEL), np.float32)
        for w in range(NW):
            gb = 8 * j - 1 + w
            col = 1024 if w == 0 else (1152 if w == 9 else 128 * (w - 1))
            if 0 <= gb < NB:
                xs[col:col + 128] = x[b, 1 + 128 * gb:1 + 128 * (gb + 1)]
        xg = x[b, [0, T - 1], :]                     # [2, D]
        qg = SCALE * (xg @ Wq.T)                     # [2, 1024]
        kg = xg @ Wk.T
        vgb = xg @ Wv.T
        # [pss, feature-row-within-tile, 2*tile + g]
        qgi = np.zeros((2, 128, 8), np.float32)
        kgi = np.zeros((2, 128, 8), np.float32)
        vgi = np.zeros((2, 34, 768), np.float32)
        for p in range(2):
            for i in range(4):
                f0 = 512 * p + 128 * i
                qgi[p, :, 2 * i:2 * i + 2] = qg[:, f0:f0 + 128].T
                kgi[p, :, 2 * i:2 * i + 2] = kg[:, f0:f0 + 128].T
            for hl in range(8):
                vgi[p, 0:2, 96 * hl:96 * hl + 64] = \
                    vgb[:, 512 * p + 64 * hl:512 * p + 64 * hl + 64]
                vgi[p, 0:2, 96 * hl + 64] = 1.0
            vgi[p, 32:34] = vgi[p, 0:2]
        in_maps.append({
            "xt": np.ascontiguousarray(xs.T).astype(BF),
            "wqk": wqk_r, "wv": wv_r, "wo": wo_r, "bo": bo,
            "maskl": zeros if j == 0 else ones,
            "maskr": zeros if j == 3 else ones,
            "ident": ident,
            "vgin": vgi.astype(BF),
            "qgin": qgi.astype(BF),
            "kgin": kgi.astype(BF),
        })
    return in_maps


def assemble_output(results, x, Wq, Wk, Wv, Wo, bo):
    x = np.asarray(x, np.float32)
    out = np.empty((B, T, D_MODEL), np.float32)
    for core in range(8):
        b, j = divmod(core, 4)
        out[b, 1 + 1024 * j:1 + 1024 * (j + 1), :] = results[core]["outt"].T

    # global token rows, exact on host
    xg = x[:, [0, T - 1], :]                      # [B, 2, D]
    qg = (xg @ Wq.T).reshape(B, 2, H, DK) * SCALE  # [B, 2, H, DK]
    kg = (xg @ Wk.T).reshape(B, 2, H, DK)
    vg = (xg @ Wv.T).reshape(B, 2, H, DK)
    for b in range(B):
        se = np.zeros((H, 2))
        wvs = np.zeros((H, 2, DK))
        for j in range(4):
            g = results[4 * b + j]["gstats"]  # [65, 32]
            for h in range(H):
                for gi in range(2):
                    se[h, gi] += g[64, 2 * h + gi]
                    wvs[h, gi] += g[0:64, 2 * h + gi]
        # add the global-key terms: scores qg . kg
        sgg = np.einsum("ghd,fhd->hgf", qg[b], kg[b])  # [H, 2g(query), 2f(key)]
        egg = np.exp(sgg)
        num = wvs + np.einsum("hgf,fhd->hgd", egg, vg[b])
        den = se + egg.sum(-1)
        og = num / den[..., None]                  # [H, 2, DK]
        for gi, trow in ((0, 0), (1, T - 1)):
            row = og[:, gi, :].reshape(H * DK)
            out[b, trow] = row @ Wo.T + bo
    return out


def kernel(x, Wq, Wk, Wv, Wo, bo):
    nc = get_nc()
    in_maps = make_inputs(x, Wq, Wk, Wv, Wo, bo)
    res = run_bass_kernel_spmd(nc, in_maps, core_ids=list(range(8)))
    LAST["res"] = res
    results = [{k: np.asarray(v) for k, v in r.items()} for r in res.results]
    return assemble_output(results, x, Wq, Wk, Wv, Wo, bo)
